# revision 3
# baseline (speedup 1.0000x reference)
"""GCN message-passing kernel for 8 Trainium2 NeuronCores (Bass/Tile).

v2 redesign vs v1 (4.66ms -> target <2.5ms):
- fp8 h table + Shared collective outputs by default (v1 had them off).
- Gathers merged per (window, stream): 26 DMAGatherAnt/layer instead of 104
  (amortizes ~1us fixed Q7 descriptor-gen cost per instruction).
- Selection matrices precomputed on host and DMA'd as inputs (removes the
  DVE IS_EQ chain, ~700us, plus GpSimd SBUF-port contention).
- Self-loop h term no longer gathered: post-activation h kept in SBUF
  (hnbS) and added to the PSUM window via identity matmuls (-5% descriptors).
- Bond-encoder matmuls window-wide (512-free) instead of per-subwindow.
- Dead-row zeroing dropped (pad edges have all-zero sel columns; pad nodes
  have deginv=0 and zero selpool rows, so garbage never propagates).
- AllReduce output Shared.
"""
import sys

sys.path.insert(0, "/opt/trn_rl_repo")

import os

import numpy as np
import ml_dtypes

import concourse.bass as bass
import concourse.bacc as bacc
import concourse.mybir as mybir
import concourse.tile as tile
from concourse.bass_utils import run_bass_kernel_spmd

P = 128
WSZ = 512          # psum node window
EPS = 1e-5
NCORES = 8
BF16 = mybir.dt.bfloat16
FP8 = mybir.dt.float8e4
F32 = mybir.dt.float32
I16 = mybir.dt.int16
S0 = 64.0          # layer-0 table scale (absorbed by BN)

USE_FP8 = os.environ.get("KGCN_FP8", "1") == "1"
HDT = FP8 if USE_FP8 else BF16
HNP = ml_dtypes.float8_e4m3fn if USE_FP8 else ml_dtypes.bfloat16


# ----------------------------------------------------------------------------
# Host preprocessing
# ----------------------------------------------------------------------------

def _wrap_idx(flat):
    n = flat.shape[0]
    assert n % 16 == 0
    w = flat.reshape(n // 16, 16).T.astype(np.int16)  # [16, n/16]
    return np.tile(w, (8, 1))


def preprocess(inputs, n_graphs=128):
    nfeat = np.asarray(inputs["nfeat"], np.int64)
    efeat = np.asarray(inputs["efeat"], np.int64)
    src = np.asarray(inputs["src"], np.int64)
    dst = np.asarray(inputs["dst"], np.int64)
    graph_ids = np.asarray(inputs["graph_ids"], np.int64)
    atom_emb = np.asarray(inputs["atom_emb"], np.float32)
    edge_emb = np.asarray(inputs["edge_emb"], np.float32)
    W = np.asarray(inputs["W"], np.float32)
    gamma = np.asarray(inputs["gamma"], np.float32)
    beta = np.asarray(inputs["beta"], np.float32)
    Wp = np.asarray(inputs["Wp"], np.float32)
    bp = np.asarray(inputs["bp"], np.float32)

    N = graph_ids.shape[0]
    E = src.shape[0]
    G = n_graphs
    GPC = G // NCORES
    AC, AV, D = atom_emb.shape
    L, BC, BV, _ = edge_emb.shape
    NCOMB = BV ** BC
    OUT = Wp.shape[1]
    HALF = NCORES // 2

    gcnt = np.bincount(graph_ids, minlength=G)
    gofs = np.concatenate([[0], np.cumsum(gcnt)])
    S = gofs[::GPC].astype(np.int64)
    assert S[-1] == N
    Nc = np.diff(S)

    NSW = int(np.ceil((Nc.max() + 1) / P))
    NPU = NSW * P
    NWIN = NPU // WSZ
    if NWIN * WSZ < NPU:
        NWIN += 1
        NPU = NWIN * WSZ
        NSW = NPU // P
    SPW = WSZ // P
    B_SPLIT = HALF * NPU
    assert B_SPLIT < 32768 and (NCORES - HALF) * NPU < 32768

    degs = np.bincount(dst, minlength=N).astype(np.float64) + 1.0
    deginv_all = (1.0 / degs).astype(np.float32)

    node_core = np.searchsorted(S[1:], np.arange(N), side="right").astype(np.int64)
    src_core = node_core[src]
    e_isL = src_core < HALF
    dLn = np.bincount(dst[e_isL], minlength=N)
    dHn = np.bincount(dst[~e_isL], minlength=N)

    # --- per-core node permutation: balance (dL, dH) across NSW bins ---
    pos_of_node = np.full(N, -1, np.int64)
    node_at_pos = [np.full(NPU, -1, np.int64) for _ in range(NCORES)]
    for c in range(NCORES):
        nodes = np.arange(S[c], S[c + 1])
        wl = dLn[nodes].astype(np.int64)
        wh = dHn[nodes].astype(np.int64)
        order = np.argsort(-(wl + wh), kind="stable")
        binL = np.zeros(NSW, np.int64)
        binH = np.zeros(NSW, np.int64)
        binN = np.zeros(NSW, np.int64)
        for i in order:
            nl, nh = wl[i], wh[i]
            cand = np.maximum(binL + nl, binH + nh) + 1e-3 * (binL + binH)
            cand[binN >= P] = 1 << 60
            b = int(np.argmin(cand))
            slot = binN[b]
            binN[b] += 1
            binL[b] += nl
            binH[b] += nh
            n = nodes[i]
            pos_of_node[n] = c * NPU + b * P + slot
            node_at_pos[c][b * P + slot] = n

    # --- edge streams per (core, stream): edges only, sorted by dst ---
    src_pg = pos_of_node[src]
    dst_pos = pos_of_node[dst]
    dst_core = node_core[dst]

    core_streams = []   # [core][stream] -> (srcpos_sorted, dstlocal_sorted)
    for c in range(NCORES):
        em = dst_core == c
        es, ed, eL = src_pg[em], dst_pos[em] - c * NPU, e_isL[em]
        per = {}
        for stream, m in (("L", eL), ("H", ~eL)):
            ssrc = es[m]
            sdst = ed[m]
            o = np.argsort(sdst, kind="stable")
            ssrc, sdst = ssrc[o], sdst[o]
            if stream == "H":
                ssrc = ssrc - B_SPLIT
            per[stream] = (ssrc, sdst)
        core_streams.append(per)

    # tiles per (sw, stream) = global max of ceil(edges_sw / P)
    TT = {}
    for stream in ("L", "H"):
        mx = 1
        for c in range(NCORES):
            _, sdst = core_streams[c][stream]
            cnt = np.bincount(sdst // P, minlength=NSW)
            mx = max(mx, int(np.ceil(cnt.max() / P)))
        TT[stream] = mx

    def pack_core(c):
        out = {}
        for stream in ("L", "H"):
            ssrc, sdst = core_streams[c][stream]
            tt = TT[stream]
            idx = np.zeros((NSW, tt * P), np.int16)   # pad -> row 0 (sel=0)
            sel = np.zeros((P, NSW * tt, P), np.float32)
            sw_of = sdst // P
            starts = np.concatenate([[0], np.cumsum(np.bincount(sw_of, minlength=NSW))])
            for sw in range(NSW):
                r0, r1 = starts[sw], starts[sw + 1]
                k = r1 - r0
                assert k <= tt * P
                idx[sw, :k] = ssrc[r0:r1]
                rows = np.arange(k)
                sel[rows % P, sw * tt + rows // P, sdst[r0:r1] - sw * P] = 1.0
            out[stream] = (idx.reshape(-1), sel.astype(HNP))
        return out

    packed = [pack_core(c) for c in range(NCORES)]

    # --- tables ---
    flat_atom = np.zeros((AC * AV + 16, D), np.float32)
    flat_atom[:AC * AV] = atom_emb.reshape(AC * AV, D) * S0
    flat_atom_q = flat_atom.astype(HNP)
    ZATOM = AC * AV

    k = np.arange(NCOMB)
    d0, d1, d2 = k // (BV * BV), (k // BV) % BV, k % BV
    T512 = edge_emb[:, 0, d0] + edge_emb[:, 1, d1] + edge_emb[:, 2, d2]
    T512[0] *= S0
    T512_q = T512.astype(HNP)

    cidx = (efeat[:, 0] * BV + efeat[:, 1]) * BV + efeat[:, 2]

    cfg = dict(N=N, E=E, G=G, GPC=GPC, D=D, L=L, OUT=OUT, NPU=NPU, NSW=NSW,
               NWIN=NWIN, SPW=SPW, TT_L=TT["L"], TT_H=TT["H"],
               B_SPLIT=B_SPLIT, NCOMB=NCOMB, AC=AC, ZATOM=ZATOM, NREAL=N)

    in_maps = []
    for c in range(NCORES):
        m = {}
        for stream in ("L", "H"):
            idx, sel = packed[c][stream]
            m[f"gidx{stream}"] = _wrap_idx(idx)
            m[f"sel{stream}"] = sel.reshape(P, -1).copy()
        em = dst_core == c
        lp = dst_pos[em] - c * NPU
        ct = np.zeros((NCOMB, NPU), np.float32)
        np.add.at(ct, (cidx[em], lp), 1.0)
        m["countT"] = ct.astype(HNP)
        dg = np.zeros(NPU, np.float32)
        rp = node_at_pos[c] >= 0
        dg[rp] = deginv_all[node_at_pos[c][rp]]
        m["deginv"] = np.tile(dg[None, :], (P, 1)).astype(ml_dtypes.bfloat16)
        sp = np.zeros((NPU, GPC), np.float32)
        gl = np.where(rp)[0]
        gid = graph_ids[node_at_pos[c][gl]] - c * GPC
        cnts = np.maximum(gcnt[c * GPC:(c + 1) * GPC], 1.0)
        sp[gl, gid] = (1.0 / cnts[gid]).astype(np.float32)
        m["selpool"] = sp
        hi = np.full((NSW, AC, P), ZATOM, np.int64)
        for st in range(NSW):
            pos = node_at_pos[c][st * P:(st + 1) * P]
            ok = pos >= 0
            nf = nfeat[pos[ok]]
            for col in range(AC):
                hi[st, col, ok] = col * AV + nf[:, col]
        m["h0idx"] = _wrap_idx(hi.reshape(-1).astype(np.int16))
        m["flat_atom"] = flat_atom_q
        m["t512"] = T512_q
        m["wl"] = W.astype(ml_dtypes.bfloat16)             # [L, D, D]
        m["gam"] = gamma.reshape(L, 1, D).copy()
        m["bet"] = beta.reshape(L, 1, D).copy()
        m["wp"] = Wp.copy()
        m["bpr"] = bp.reshape(1, OUT).copy()
        m["ident"] = np.eye(P, dtype=HNP)
        in_maps.append(m)

    meta = dict(S=S, Nc=Nc)
    return cfg, in_maps, meta


# ----------------------------------------------------------------------------
# Device kernel builder (uniform SPMD program)
# ----------------------------------------------------------------------------

def build(cfg):
    D = cfg["D"]; L = cfg["L"]; NPU = cfg["NPU"]; NSW = cfg["NSW"]
    NWIN = cfg["NWIN"]; SPW = cfg["SPW"]; TT_L = cfg["TT_L"]; TT_H = cfg["TT_H"]
    NCOMB = cfg["NCOMB"]; AC = cfg["AC"]; GPC = cfg["GPC"]; OUT = cfg["OUT"]
    B_SPLIT = cfg["B_SPLIT"]; NREAL = cfg["NREAL"]
    KD = D // P
    NKC = NCOMB // P
    NT_L, NT_H = NSW * TT_L, NSW * TT_H
    WT_L, WT_H = SPW * TT_L, SPW * TT_H     # gather tiles per window
    WT_A = SPW * AC                          # h0 gather tiles per window
    GT_SL = max(WT_L, WT_A)
    NQ = int(os.environ.get("KGCN_NQ", "4"))

    nc = bacc.Bacc("TRN2", target_bir_lowering=False, debug=False,
                   num_devices=NCORES, num_swdge_queues=NQ)

    def allgather(ins, outs):
        if USE_FP8:
            ins = [ap.bitcast(BF16) for ap in ins]
            outs = [ap.bitcast(BF16) for ap in outs]
        nc.gpsimd.collective_compute(
            "AllGather", mybir.AluOpType.bypass,
            replica_groups=[list(range(NCORES))], ins=ins, outs=outs)

    def allreduce(ins, outs):
        nc.gpsimd.collective_compute(
            "AllReduce", mybir.AluOpType.add,
            replica_groups=[list(range(NCORES))], ins=ins, outs=outs)

    t_gidxL = nc.dram_tensor("gidxL", [P, NT_L * P // 16], I16, kind="ExternalInput")
    t_gidxH = nc.dram_tensor("gidxH", [P, NT_H * P // 16], I16, kind="ExternalInput")
    t_selL = nc.dram_tensor("selL", [P, NT_L * P], HDT, kind="ExternalInput")
    t_selH = nc.dram_tensor("selH", [P, NT_H * P], HDT, kind="ExternalInput")
    t_countT = nc.dram_tensor("countT", [NCOMB, NPU], HDT, kind="ExternalInput")
    t_deginv = nc.dram_tensor("deginv", [P, NPU], BF16, kind="ExternalInput")
    t_selpool = nc.dram_tensor("selpool", [NPU, GPC], F32, kind="ExternalInput")
    t_h0idx = nc.dram_tensor("h0idx", [P, NSW * AC * P // 16], I16, kind="ExternalInput")
    t_atom = nc.dram_tensor("flat_atom", [AC * 128 + 16, D], HDT, kind="ExternalInput")
    t_t512 = nc.dram_tensor("t512", [L, NCOMB, D], HDT, kind="ExternalInput")
    t_wl = nc.dram_tensor("wl", [L, D, D], BF16, kind="ExternalInput")
    t_gam = nc.dram_tensor("gam", [L, 1, D], F32, kind="ExternalInput")
    t_bet = nc.dram_tensor("bet", [L, 1, D], F32, kind="ExternalInput")
    t_wp = nc.dram_tensor("wp", [D, OUT], F32, kind="ExternalInput")
    t_bp = nc.dram_tensor("bpr", [1, OUT], F32, kind="ExternalInput")
    t_ident = nc.dram_tensor("ident", [P, P], HDT, kind="ExternalInput")
    t_out = nc.dram_tensor("out_g", [GPC, OUT], F32, kind="ExternalOutput")
    t_hfull = nc.dram_tensor("h_full", [NCORES * NPU, D], HDT, addr_space="Shared")
    t_hnew = nc.dram_tensor("h_newc", [NPU, D], HDT)
    t_arin = [nc.dram_tensor(f"arin{l}", [2, D], F32) for l in range(L)]
    t_arout = [nc.dram_tensor(f"arout{l}", [2, D], F32, addr_space="Shared")
               for l in range(L)]

    qrr = [0]

    def next_q():
        q = qrr[0]
        qrr[0] = (qrr[0] + 1) % NQ
        return q

    with tile.TileContext(nc) as tc:
        with (
            tc.tile_pool(name="static", bufs=1) as stp,
            tc.tile_pool(name="gath", bufs=2) as gpool,
            tc.tile_pool(name="selp", bufs=2) as selpool_p,
            tc.tile_pool(name="xt", bufs=3) as xtp,
            tc.tile_pool(name="work", bufs=3) as wk,
            tc.tile_pool(name="small", bufs=1) as smp,
            tc.tile_pool(name="winps", bufs=2, space="PSUM") as wps,
            tc.tile_pool(name="hlps", bufs=1, space="PSUM") as hps,
            tc.tile_pool(name="smps", bufs=1, space="PSUM") as sps,
            tc.tile_pool(name="abps", bufs=1, space="PSUM") as aps,
        ):
            # ---- static SBUF preloads ----
            gidxL = stp.tile([P, NT_L * P // 16], I16)
            gidxH = stp.tile([P, NT_H * P // 16], I16)
            h0idx = stp.tile([P, NSW * AC * P // 16], I16)
            selpS = stp.tile([P, NSW, GPC], F32)
            dgS = stp.tile([P, NPU], BF16)
            wS = stp.tile([P, L, KD, D], BF16)
            t5S = stp.tile([P, L, NKC, D], HDT)
            gamS = stp.tile([1, L, D], F32)
            betS = stp.tile([1, L, D], F32)
            wpS = stp.tile([P, KD, OUT], F32)
            bpS = stp.tile([1, OUT], F32)
            onesS = stp.tile([1, P], F32)
            onecol = stp.tile([P, 1], BF16)
            identS = stp.tile([P, P], HDT)
            hlinS = stp.tile([P, NSW, D], BF16)
            hnbS = stp.tile([P, NSW, D], HDT)
            epsS = stp.tile([1, 1], F32)
            nc.vector.memset(epsS[:], EPS)
            nc.sync.dma_start(gidxL[:], t_gidxL[:])
            nc.sync.dma_start(gidxH[:], t_gidxH[:])
            nc.sync.dma_start(h0idx[:], t_h0idx[:])
            nc.sync.dma_start(identS[:], t_ident[:])
            nc.sync.dma_start(selpS[:], t_selpool.ap().rearrange("(s p) g -> p s g", p=P))
            nc.sync.dma_start(dgS[:], t_deginv[:])
            nc.sync.dma_start(wS[:], t_wl.ap().rearrange("l (k p) d -> p l k d", p=P))
            nc.sync.dma_start(t5S[:], t_t512.ap().rearrange("l (k p) d -> p l k d", p=P))
            nc.sync.dma_start(gamS[:], t_gam.ap().rearrange("l o d -> o l d"))
            nc.sync.dma_start(betS[:], t_bet.ap().rearrange("l o d -> o l d"))
            nc.sync.dma_start(wpS[:], t_wp.ap().rearrange("(k p) o -> p k o", p=P))
            nc.sync.dma_start(bpS[:], t_bp[:])
            nc.vector.memset(onesS[:], 1.0)
            nc.vector.memset(onecol[:], 1.0)

            # ================= h0: atom embedding sums =================
            for w in range(NWIN):
                nidx = WT_A * P
                g = gpool.tile([P, GT_SL, D], HDT, tag="gL", name="gLt")
                nc.gpsimd.dma_gather(
                    g[:, 0:WT_A, :], t_atom[:],
                    h0idx[:, w * (nidx // 16):(w + 1) * (nidx // 16)],
                    nidx, nidx, D, single_packet=False, queue_num=next_q())
                for sw in range(SPW):
                    st = w * SPW + sw
                    a0 = sw * AC
                    acc = wk.tile([P, 4, D], BF16, tag="h0acc", bufs=2)
                    nc.vector.tensor_tensor(out=acc[:], in0=g[:, a0:a0 + 4, :],
                                            in1=g[:, a0 + 4:a0 + 8, :],
                                            op=mybir.AluOpType.add)
                    acc2 = wk.tile([P, 2, D], BF16, tag="h0acc2", bufs=2)
                    nc.vector.tensor_tensor(out=acc2[:], in0=acc[:, 0:2, :],
                                            in1=acc[:, 2:4, :],
                                            op=mybir.AluOpType.add)
                    h0t = wk.tile([P, D], BF16, tag="h0t", bufs=2)
                    nc.vector.tensor_tensor(out=h0t[:], in0=acc2[:, 0, :],
                                            in1=acc2[:, 1, :],
                                            op=mybir.AluOpType.add)
                    nc.vector.tensor_tensor(out=hnbS[:, st, :], in0=h0t[:],
                                            in1=g[:, a0 + 8, :],
                                            op=mybir.AluOpType.add)
                    nc.sync.dma_start(t_hnew[st * P:(st + 1) * P, :],
                                      hnbS[:, st, :])
            allgather([t_hnew[:]], [t_hfull[:]])

            # ================= layers =================
            for l in range(L):
                stats0 = sps.tile([1, D], F32, tag="stats0")
                stats1 = sps.tile([1, D], F32, tag="stats1")
                if l == L - 1:
                    poolps = [sps.tile([P, GPC], F32, tag=f"pool{h}",
                                       name=f"pool{h}") for h in range(KD)]
                for w in range(NWIN):
                    winp = [wps.tile([P, WSZ], F32, tag="win", name=f"win{h}")
                            for h in range(KD)]
                    ctk = wk.tile([P, NKC, WSZ], HDT, tag="ct", bufs=2)
                    nc.sync.dma_start(
                        ctk[:], t_countT.ap().rearrange(
                            "(k p) n -> p k n", p=P)[:, :, w * WSZ:(w + 1) * WSZ])
                    gts = {}
                    sels = {}
                    for stream, wt, gidx, tsel in (
                            ("L", WT_L, gidxL, t_selL),
                            ("H", WT_H, gidxH, t_selH)):
                        nidx = wt * P
                        gt = gpool.tile([P, GT_SL if stream == "L" else WT_H, D],
                                        HDT, tag=f"g{stream}",
                                        name=f"g{stream}t")
                        tbl = (t_hfull[0:B_SPLIT, :] if stream == "L"
                               else t_hfull[B_SPLIT:NCORES * NPU, :])
                        nc.gpsimd.dma_gather(
                            gt[:, 0:wt, :], tbl,
                            gidx[:, w * (nidx // 16):(w + 1) * (nidx // 16)],
                            nidx, nidx, D, single_packet=False,
                            queue_num=next_q())
                        sel = selpool_p.tile([P, wt, P], HDT, tag=f"s{stream}",
                                             name=f"s{stream}t")
                        nc.sync.dma_start(
                            sel[:], tsel[:, w * (wt * P):(w + 1) * (wt * P)])
                        gts[stream] = gt
                        sels[stream] = sel
                    # bond term: window-wide, starts the PSUM accumulation
                    for kk in range(NKC):
                        for h in range(KD):
                            nc.tensor.matmul(
                                out=winp[h][:],
                                lhsT=t5S[:, l, kk, h * P:(h + 1) * P],
                                rhs=ctk[:, kk, :],
                                start=(kk == 0), stop=False)
                    for sw in range(SPW):
                        st = w * SPW + sw
                        # self term via identity (hnbS holds this layer's input)
                        for h in range(KD):
                            nc.tensor.matmul(
                                out=winp[h][:, sw * P:(sw + 1) * P],
                                lhsT=hnbS[:, st, h * P:(h + 1) * P],
                                rhs=identS[:],
                                start=False, stop=False)
                        for stream, tt in (("L", TT_L), ("H", TT_H)):
                            gt = gts[stream]
                            sel = sels[stream]
                            last_stream = stream == "H"
                            for t in range(tt):
                                ti = sw * tt + t
                                for h in range(KD):
                                    nc.tensor.matmul(
                                        out=winp[h][:, sw * P:(sw + 1) * P],
                                        lhsT=gt[:, ti, h * P:(h + 1) * P],
                                        rhs=sel[:, ti, :],
                                        start=False,
                                        stop=(last_stream and t == tt - 1))
                    # x^T = deginv * window  (bf16)
                    xt = [xtp.tile([P, WSZ], BF16, tag="xt", name=f"xt{h}")
                          for h in range(KD)]
                    for h in range(KD):
                        nc.vector.tensor_tensor(
                            out=xt[h][:], in0=winp[h][:],
                            in1=dgS[:, w * WSZ:(w + 1) * WSZ],
                            op=mybir.AluOpType.mult)
                    # update matmul + stats per subtile
                    for sw in range(SPW):
                        st = w * SPW + sw
                        hlp = hps.tile([P, D], F32, tag="hl")
                        for h in range(KD):
                            nc.tensor.matmul(
                                out=hlp[:],
                                lhsT=xt[h][:, sw * P:(sw + 1) * P],
                                rhs=wS[:, l, h, :],
                                start=(h == 0), stop=(h == KD - 1))
                        nc.scalar.activation(hlinS[:, st, :], hlp[:],
                                             mybir.ActivationFunctionType.Copy)
                        sq = wk.tile([P, D], BF16, tag="sq")
                        nc.vector.tensor_tensor(out=sq[:], in0=hlinS[:, st, :],
                                                in1=hlinS[:, st, :],
                                                op=mybir.AluOpType.mult)
                        nc.tensor.matmul(out=stats0[:],
                                         lhsT=onecol[:],
                                         rhs=hlinS[:, st, :],
                                         start=(st == 0), stop=(st == NSW - 1))
                        nc.tensor.matmul(out=stats1[:],
                                         lhsT=onecol[:], rhs=sq[:],
                                         start=(st == 0), stop=(st == NSW - 1))
                # --- BN stats allreduce + scale/shift ---
                stsb0 = smp.tile([1, D], F32, tag="stsb0")
                stsb1 = smp.tile([1, D], F32, tag="stsb1")
                nc.scalar.activation(stsb0[:], stats0[:],
                                     mybir.ActivationFunctionType.Copy)
                nc.scalar.activation(stsb1[:], stats1[:],
                                     mybir.ActivationFunctionType.Copy)
                nc.sync.dma_start(t_arin[l][0:1, :], stsb0[:])
                nc.sync.dma_start(t_arin[l][1:2, :], stsb1[:])
                allreduce([t_arin[l][:]], [t_arout[l][:]])
                stg0 = smp.tile([1, D], F32, tag="stg0")
                stg1 = smp.tile([1, D], F32, tag="stg1")
                nc.sync.dma_start(stg0[:], t_arout[l][0:1, :])
                nc.sync.dma_start(stg1[:], t_arout[l][1:2, :])
                mean = smp.tile([1, D], F32, tag="mean")
                nc.vector.tensor_scalar_mul(mean[:], stg0[:], 1.0 / NREAL)
                msq = smp.tile([1, D], F32, tag="msq")
                nc.vector.tensor_scalar_mul(msq[:], stg1[:], 1.0 / NREAL)
                var = smp.tile([1, D], F32, tag="var")
                nc.vector.tensor_tensor(out=var[:], in0=mean[:], in1=mean[:],
                                        op=mybir.AluOpType.mult)
                nc.vector.tensor_tensor(out=var[:], in0=msq[:], in1=var[:],
                                        op=mybir.AluOpType.subtract)
                sd = smp.tile([1, D], F32, tag="sd")
                nc.scalar.activation(sd[:], var[:],
                                     mybir.ActivationFunctionType.Sqrt,
                                     bias=epsS[:])
                rsq = smp.tile([1, D], F32, tag="rsq")
                nc.vector.reciprocal(rsq[:], sd[:])
                scl = smp.tile([1, D], F32, tag="scl")
                nc.vector.tensor_tensor(out=scl[:], in0=rsq[:],
                                        in1=gamS[:, l, :],
                                        op=mybir.AluOpType.mult)
                sft = smp.tile([1, D], F32, tag="sft")
                nc.vector.tensor_tensor(out=sft[:], in0=mean[:], in1=scl[:],
                                        op=mybir.AluOpType.mult)
                nc.vector.tensor_tensor(out=sft[:], in0=betS[:, l, :],
                                        in1=sft[:],
                                        op=mybir.AluOpType.subtract)
                ab = aps.tile([P, 2 * D], F32, tag="ab")
                nc.tensor.matmul(out=ab[:, 0:D], lhsT=onesS[:], rhs=scl[:],
                                 start=True, stop=True)
                nc.tensor.matmul(out=ab[:, D:2 * D], lhsT=onesS[:], rhs=sft[:],
                                 start=True, stop=True)
                # --- apply + (layer L-1) pooling ---
                for st in range(NSW):
                    hnf = wk.tile([P, D], F32, tag="hnf")
                    nc.vector.tensor_tensor(out=hnf[:], in0=hlinS[:, st, :],
                                            in1=ab[:, 0:D],
                                            op=mybir.AluOpType.mult)
                    nc.vector.tensor_tensor(out=hnf[:], in0=hnf[:],
                                            in1=ab[:, D:2 * D],
                                            op=mybir.AluOpType.add)
                    if l < L - 1:
                        nc.scalar.activation(hnbS[:, st, :], hnf[:],
                                             mybir.ActivationFunctionType.Relu)
                        nc.sync.dma_start(t_hnew[st * P:(st + 1) * P, :],
                                          hnbS[:, st, :])
                    else:
                        hnr = wk.tile([P, D], F32, tag="hnr")
                        nc.vector.tensor_scalar_max(hnr[:], hnf[:], 0.0)
                        for h in range(KD):
                            nc.tensor.matmul(
                                out=poolps[h][:],
                                lhsT=hnr[:, h * P:(h + 1) * P],
                                rhs=selpS[:, st, :],
                                start=(st == 0), stop=(st == NSW - 1))
                if l < L - 1:
                    allgather([t_hnew[:]], [t_hfull[:]])

            # ================= readout =================
            gts = smp.tile([P, KD * GPC], F32, tag="gts")
            for h in range(KD):
                nc.scalar.activation(gts[:, h * GPC:(h + 1) * GPC],
                                     poolps[h][:],
                                     mybir.ActivationFunctionType.Copy)
            ones16 = smp.tile([1, GPC], F32, tag="o16")
            nc.vector.memset(ones16[:], 1.0)
            outp = sps.tile([GPC, OUT], F32, tag="stats0")
            for h in range(KD):
                nc.tensor.matmul(out=outp[:],
                                 lhsT=gts[:, h * GPC:(h + 1) * GPC],
                                 rhs=wpS[:, h, :], start=(h == 0), stop=False)
            nc.tensor.matmul(out=outp[:], lhsT=ones16[:], rhs=bpS[:],
                             start=False, stop=True)
            outs = smp.tile([GPC, OUT], F32, tag="outs")
            nc.scalar.activation(outs[:], outp[:],
                                 mybir.ActivationFunctionType.Copy)
            nc.sync.dma_start(t_out[:], outs[:])

    nc.compile()
    return nc


LAST = {}


def kernel(**inputs):
    cfg, in_maps, _ = preprocess(inputs)
    nc = build(cfg)
    trace = os.environ.get("KGCN_TRACE") == "1"
    res = run_bass_kernel_spmd(nc, in_maps, list(range(NCORES)), trace=trace)
    LAST["exec_time_ns"] = res.exec_time_ns
    LAST["profile_json"] = res.profile_json
    out = np.concatenate([res.results[c]["out_g"] for c in range(NCORES)], 0)
    return out.astype(np.float32)


if __name__ == "__main__":
    pass


# revision 23
# speedup vs baseline: 1.2042x; 1.2042x over previous
"""GCN message-passing kernel for 8 Trainium2 NeuronCores (Bass/Tile).

v2 redesign vs v1 (4.66ms -> target <2.5ms):
- fp8 h table + Shared collective outputs by default (v1 had them off).
- Gathers merged per (window, stream): 26 DMAGatherAnt/layer instead of 104
  (amortizes ~1us fixed Q7 descriptor-gen cost per instruction).
- Selection matrices precomputed on host and DMA'd as inputs (removes the
  DVE IS_EQ chain, ~700us, plus GpSimd SBUF-port contention).
- Self-loop h term no longer gathered: post-activation h kept in SBUF
  (hnbS) and added to the PSUM window via identity matmuls (-5% descriptors).
- Bond-encoder matmuls window-wide (512-free) instead of per-subwindow.
- Dead-row zeroing dropped (pad edges have all-zero sel columns; pad nodes
  have deginv=0 and zero selpool rows, so garbage never propagates).
- AllReduce output Shared.
"""
import sys

sys.path.insert(0, "/opt/trn_rl_repo")

import os

import numpy as np
import ml_dtypes

import concourse.bass as bass
import concourse.bacc as bacc
import concourse.mybir as mybir
import concourse.tile as tile
from concourse.bass_utils import run_bass_kernel_spmd

P = 128
WSZ = 512          # psum node window
EPS = 1e-5
NCORES = 8
BF16 = mybir.dt.bfloat16
FP8 = mybir.dt.float8e4
F32 = mybir.dt.float32
I16 = mybir.dt.int16
S0 = 64.0          # layer-0 table scale (absorbed by BN)

USE_FP8 = os.environ.get("KGCN_FP8", "1") == "1"
HDT = FP8 if USE_FP8 else BF16
HNP = ml_dtypes.float8_e4m3fn if USE_FP8 else ml_dtypes.bfloat16


# ----------------------------------------------------------------------------
# Host preprocessing
# ----------------------------------------------------------------------------

def _wrap_idx(flat):
    n = flat.shape[0]
    assert n % 16 == 0
    w = flat.reshape(n // 16, 16).T.astype(np.int16)  # [16, n/16]
    return np.tile(w, (8, 1))


def preprocess(inputs, n_graphs=128):
    nfeat = np.asarray(inputs["nfeat"], np.int64)
    efeat = np.asarray(inputs["efeat"], np.int64)
    src = np.asarray(inputs["src"], np.int64)
    dst = np.asarray(inputs["dst"], np.int64)
    graph_ids = np.asarray(inputs["graph_ids"], np.int64)
    atom_emb = np.asarray(inputs["atom_emb"], np.float32)
    edge_emb = np.asarray(inputs["edge_emb"], np.float32)
    W = np.asarray(inputs["W"], np.float32)
    gamma = np.asarray(inputs["gamma"], np.float32)
    beta = np.asarray(inputs["beta"], np.float32)
    Wp = np.asarray(inputs["Wp"], np.float32)
    bp = np.asarray(inputs["bp"], np.float32)

    N = graph_ids.shape[0]
    E = src.shape[0]
    G = n_graphs
    GPC = G // NCORES
    AC, AV, D = atom_emb.shape
    L, BC, BV, _ = edge_emb.shape
    NCOMB = BV ** BC
    OUT = Wp.shape[1]
    HALF = NCORES // 2

    gcnt = np.bincount(graph_ids, minlength=G)
    gofs = np.concatenate([[0], np.cumsum(gcnt)])
    S = gofs[::GPC].astype(np.int64)
    assert S[-1] == N
    Nc = np.diff(S)

    NSW = int(np.ceil((Nc.max() + 1) / P))
    NPU = NSW * P
    NWIN = NPU // WSZ
    if NWIN * WSZ < NPU:
        NWIN += 1
        NPU = NWIN * WSZ
        NSW = NPU // P
    SPW = WSZ // P
    B_SPLIT = HALF * NPU
    assert B_SPLIT < 32768 and (NCORES - HALF) * NPU < 32768

    degs = np.bincount(dst, minlength=N).astype(np.float64) + 1.0
    deginv_all = (1.0 / degs).astype(np.float32)

    node_core = np.searchsorted(S[1:], np.arange(N), side="right").astype(np.int64)
    src_core = node_core[src]
    e_isL = src_core < HALF
    dLn = np.bincount(dst[e_isL], minlength=N)
    dHn = np.bincount(dst[~e_isL], minlength=N)

    # --- per-core node permutation: balance (dL, dH) across NSW bins ---
    pos_of_node = np.full(N, -1, np.int64)
    node_at_pos = [np.full(NPU, -1, np.int64) for _ in range(NCORES)]
    for c in range(NCORES):
        nodes = np.arange(S[c], S[c + 1])
        wl = dLn[nodes].astype(np.int64)
        wh = dHn[nodes].astype(np.int64)
        order = np.argsort(-(wl + wh), kind="stable")
        binL = np.zeros(NSW, np.int64)
        binH = np.zeros(NSW, np.int64)
        binN = np.zeros(NSW, np.int64)
        for i in order:
            nl, nh = wl[i], wh[i]
            cand = np.maximum(binL + nl, binH + nh) + 1e-3 * (binL + binH)
            cand[binN >= P] = 1 << 60
            b = int(np.argmin(cand))
            slot = binN[b]
            binN[b] += 1
            binL[b] += nl
            binH[b] += nh
            n = nodes[i]
            pos_of_node[n] = c * NPU + b * P + slot
            node_at_pos[c][b * P + slot] = n

    # --- edge streams per (core, stream): edges only, sorted by dst ---
    src_pg = pos_of_node[src]
    dst_pos = pos_of_node[dst]
    dst_core = node_core[dst]

    core_streams = []   # [core][stream] -> (srcpos_sorted, dstlocal_sorted)
    for c in range(NCORES):
        em = dst_core == c
        es, ed, eL = src_pg[em], dst_pos[em] - c * NPU, e_isL[em]
        per = {}
        for stream, m in (("L", eL), ("H", ~eL)):
            ssrc = es[m]
            sdst = ed[m]
            o = np.argsort(sdst, kind="stable")
            ssrc, sdst = ssrc[o], sdst[o]
            if stream == "H":
                ssrc = ssrc - B_SPLIT
            per[stream] = (ssrc, sdst)
        core_streams.append(per)

    # tiles per (sw, stream) = global max of ceil(edges_sw / P)
    TT = {}
    for stream in ("L", "H"):
        mx = 1
        for c in range(NCORES):
            _, sdst = core_streams[c][stream]
            cnt = np.bincount(sdst // P, minlength=NSW)
            mx = max(mx, int(np.ceil(cnt.max() / P)))
        TT[stream] = mx

    def pack_core(c):
        out = {}
        for stream in ("L", "H"):
            ssrc, sdst = core_streams[c][stream]
            tt = TT[stream]
            idx = np.zeros((NSW, tt * P), np.int16)   # pad -> row 0 (sel=0)
            sel = np.zeros((P, NSW * tt, P), np.float32)
            sw_of = sdst // P
            starts = np.concatenate([[0], np.cumsum(np.bincount(sw_of, minlength=NSW))])
            for sw in range(NSW):
                r0, r1 = starts[sw], starts[sw + 1]
                k = r1 - r0
                assert k <= tt * P
                idx[sw, :k] = ssrc[r0:r1]
                rows = np.arange(k)
                sel[rows % P, sw * tt + rows // P, sdst[r0:r1] - sw * P] = 1.0
            out[stream] = (idx.reshape(-1), sel.astype(HNP))
        return out

    packed = [pack_core(c) for c in range(NCORES)]

    # --- tables ---
    # atom9[p, a, :] = atom_emb[a, p, :] * S0  (vocab entry p of column a)
    atom9_q = (np.transpose(atom_emb, (1, 0, 2)) * S0).astype(HNP)

    k = np.arange(NCOMB)
    d0, d1, d2 = k // (BV * BV), (k // BV) % BV, k % BV
    T512 = edge_emb[:, 0, d0] + edge_emb[:, 1, d1] + edge_emb[:, 2, d2]
    T512[0] *= S0
    T512_q = T512.astype(HNP)

    cidx = (efeat[:, 0] * BV + efeat[:, 1]) * BV + efeat[:, 2]

    cfg = dict(N=N, E=E, G=G, GPC=GPC, D=D, L=L, OUT=OUT, NPU=NPU, NSW=NSW,
               NWIN=NWIN, SPW=SPW, TT_L=TT["L"], TT_H=TT["H"],
               B_SPLIT=B_SPLIT, NCOMB=NCOMB, AC=AC, NREAL=N)

    in_maps = []
    for c in range(NCORES):
        m = {}
        for stream in ("L", "H"):
            idx, sel = packed[c][stream]
            m[f"gidx{stream}"] = _wrap_idx(idx)
            m[f"sel{stream}"] = sel.reshape(P, -1).copy()
        em = dst_core == c
        lp = dst_pos[em] - c * NPU
        ct = np.zeros((NCOMB, NPU), np.float32)
        np.add.at(ct, (cidx[em], lp), 1.0)
        m["countT"] = ct.astype(HNP)
        dg = np.zeros(NPU, np.float32)
        rp = node_at_pos[c] >= 0
        dg[rp] = deginv_all[node_at_pos[c][rp]]
        m["deginv"] = np.tile(dg[None, :], (P, 1)).astype(ml_dtypes.bfloat16)
        sp = np.zeros((NPU, GPC), np.float32)
        gl = np.where(rp)[0]
        gid = graph_ids[node_at_pos[c][gl]] - c * GPC
        cnts = np.maximum(gcnt[c * GPC:(c + 1) * GPC], 1.0)
        sp[gl, gid] = (1.0 / cnts[gid]).astype(np.float32)
        m["selpool"] = sp
        cnt9 = np.zeros((P, AC, NPU), HNP)
        pos_r = np.where(rp)[0]
        nf = nfeat[node_at_pos[c][pos_r]]
        for a in range(AC):
            cnt9[nf[:, a], a, pos_r] = 1.0
        m["cnt9"] = cnt9.reshape(P, -1).copy()
        m["atom9"] = atom9_q
        m["t512"] = T512_q
        m["wl"] = W.astype(ml_dtypes.bfloat16)             # [L, D, D]
        m["gam"] = gamma.reshape(L, 1, D).copy()
        m["bet"] = beta.reshape(L, 1, D).copy()
        m["wp"] = Wp.copy()
        m["bpr"] = bp.reshape(1, OUT).copy()
        m["ident"] = np.eye(P, dtype=HNP)
        in_maps.append(m)

    meta = dict(S=S, Nc=Nc)
    return cfg, in_maps, meta


# ----------------------------------------------------------------------------
# Device kernel builder (uniform SPMD program)
# ----------------------------------------------------------------------------

def build(cfg):
    D = cfg["D"]; L = cfg["L"]; NPU = cfg["NPU"]; NSW = cfg["NSW"]
    NWIN = cfg["NWIN"]; SPW = cfg["SPW"]; TT_L = cfg["TT_L"]; TT_H = cfg["TT_H"]
    NCOMB = cfg["NCOMB"]; AC = cfg["AC"]; GPC = cfg["GPC"]; OUT = cfg["OUT"]
    B_SPLIT = cfg["B_SPLIT"]; NREAL = cfg["NREAL"]
    KD = D // P
    NKC = NCOMB // P
    NT_L, NT_H = NSW * TT_L, NSW * TT_H
    WT_L, WT_H = SPW * TT_L, SPW * TT_H     # gather tiles per window
    NQ = int(os.environ.get("KGCN_NQ", "4"))
    PREP = os.environ.get("KGCN_PREP", "1") == "1"
    NPRE = 3                                 # windows prepped ahead at layer start

    nc = bacc.Bacc("TRN2", target_bir_lowering=False, debug=False,
                   num_devices=NCORES, num_swdge_queues=NQ)

    def allgather(ins, outs):
        if USE_FP8:
            ins = [ap.bitcast(BF16) for ap in ins]
            outs = [ap.bitcast(BF16) for ap in outs]
        nc.gpsimd.collective_compute(
            "AllGather", mybir.AluOpType.bypass,
            replica_groups=[list(range(NCORES))], ins=ins, outs=outs)

    def allreduce(ins, outs):
        nc.gpsimd.collective_compute(
            "AllReduce", mybir.AluOpType.add,
            replica_groups=[list(range(NCORES))], ins=ins, outs=outs)

    t_gidxL = nc.dram_tensor("gidxL", [P, NT_L * P // 16], I16, kind="ExternalInput")
    t_gidxH = nc.dram_tensor("gidxH", [P, NT_H * P // 16], I16, kind="ExternalInput")
    t_selL = nc.dram_tensor("selL", [P, NT_L * P], HDT, kind="ExternalInput")
    t_selH = nc.dram_tensor("selH", [P, NT_H * P], HDT, kind="ExternalInput")
    t_countT = nc.dram_tensor("countT", [NCOMB, NPU], HDT, kind="ExternalInput")
    t_deginv = nc.dram_tensor("deginv", [P, NPU], BF16, kind="ExternalInput")
    t_selpool = nc.dram_tensor("selpool", [NPU, GPC], F32, kind="ExternalInput")
    t_cnt9 = nc.dram_tensor("cnt9", [P, AC * NPU], HDT, kind="ExternalInput")
    t_atom9 = nc.dram_tensor("atom9", [P, AC, D], HDT, kind="ExternalInput")
    t_t512 = nc.dram_tensor("t512", [L, NCOMB, D], HDT, kind="ExternalInput")
    t_wl = nc.dram_tensor("wl", [L, D, D], BF16, kind="ExternalInput")
    t_gam = nc.dram_tensor("gam", [L, 1, D], F32, kind="ExternalInput")
    t_bet = nc.dram_tensor("bet", [L, 1, D], F32, kind="ExternalInput")
    t_wp = nc.dram_tensor("wp", [D, OUT], F32, kind="ExternalInput")
    t_bp = nc.dram_tensor("bpr", [1, OUT], F32, kind="ExternalInput")
    t_ident = nc.dram_tensor("ident", [P, P], HDT, kind="ExternalInput")
    t_out = nc.dram_tensor("out_g", [GPC, OUT], F32, kind="ExternalOutput")
    t_hfull = nc.dram_tensor("h_full", [NCORES * NPU, D], HDT, addr_space="Shared")
    t_hnew = nc.dram_tensor("h_newc", [NPU, D], HDT)
    t_arin = [nc.dram_tensor(f"arin{l}", [2, D], F32) for l in range(L)]
    t_arout = [nc.dram_tensor(f"arout{l}", [2, D], F32, addr_space="Shared")
               for l in range(L)]

    dma_sems = [nc.alloc_semaphore(f"swdge_dma{q}") for q in range(NQ)]

    def wqueues(w):
        qa = (2 * w) % NQ
        return qa, qa + 1

    with tile.TileContext(nc) as tc:
        with (
            tc.tile_pool(name="static", bufs=1) as stp,
            tc.tile_pool(name="gath", bufs=3) as gpool,
            tc.tile_pool(name="selp", bufs=2) as selpool_p,
            tc.tile_pool(name="xt", bufs=2) as xtp,
            tc.tile_pool(name="work", bufs=3) as wk,
            tc.tile_pool(name="small", bufs=1) as smp,
            tc.tile_pool(name="winps", bufs=2, space="PSUM") as wps,
            tc.tile_pool(name="hlps", bufs=1, space="PSUM") as hps,
            tc.tile_pool(name="smps", bufs=1, space="PSUM") as sps,
            tc.tile_pool(name="abps", bufs=1, space="PSUM") as aps,
        ):
            # ---- static SBUF preloads ----
            atom9S = stp.tile([P, AC, D], HDT)
            selpS = stp.tile([P, NSW, GPC], F32)
            dgS = stp.tile([P, NPU], BF16)
            wS = stp.tile([P, L, KD, D], BF16)
            t5S = stp.tile([P, L, NKC, D], HDT)
            gamS = stp.tile([1, L, D], F32)
            betS = stp.tile([1, L, D], F32)
            wpS = stp.tile([P, KD, OUT], F32)
            bpS = stp.tile([1, OUT], F32)
            onesS = stp.tile([1, P], F32)
            onecol = stp.tile([P, 1], BF16)
            identS = stp.tile([P, P], HDT)
            hlinS = stp.tile([P, NSW, D], BF16)
            hnbS = stp.tile([P, NSW, D], HDT)
            epsS = stp.tile([1, 1], F32)
            nc.vector.memset(epsS[:], EPS)
            nc.sync.dma_start(atom9S[:], t_atom9[:])
            nc.sync.dma_start(identS[:], t_ident[:])
            nc.sync.dma_start(selpS[:], t_selpool.ap().rearrange("(s p) g -> p s g", p=P))
            nc.sync.dma_start(dgS[:], t_deginv[:])
            nc.sync.dma_start(wS[:], t_wl.ap().rearrange("l (k p) d -> p l k d", p=P))
            nc.sync.dma_start(t5S[:], t_t512.ap().rearrange("l (k p) d -> p l k d", p=P))
            nc.sync.dma_start(gamS[:], t_gam.ap().rearrange("l o d -> o l d"))
            nc.sync.dma_start(betS[:], t_bet.ap().rearrange("l o d -> o l d"))
            nc.sync.dma_start(wpS[:], t_wp.ap().rearrange("(k p) o -> p k o", p=P))
            nc.sync.dma_start(bpS[:], t_bp[:])
            nc.vector.memset(onesS[:], 1.0)
            nc.vector.memset(onecol[:], 1.0)

            # ============ h0: atom embedding sums via count matmuls ============
            for w in range(NWIN):
                cnt = wk.tile([P, AC, WSZ], HDT, tag="cnt", bufs=2)
                nc.sync.dma_start(
                    cnt[:], t_cnt9.ap().rearrange("p (a n) -> p a n", a=AC)
                    [:, :, w * WSZ:(w + 1) * WSZ])
                for sw in range(SPW):
                    st = w * SPW + sw
                    h0p = hps.tile([P, D], F32, tag="hl")
                    for a in range(AC):
                        nc.tensor.matmul(
                            out=h0p[:],
                            lhsT=cnt[:, a, sw * P:(sw + 1) * P],
                            rhs=atom9S[:, a, :],
                            start=(a == 0), stop=(a == AC - 1))
                    nc.scalar.activation(hnbS[:, st, :], h0p[:],
                                         mybir.ActivationFunctionType.Copy)
                    nc.sync.dma_start(t_hnew[st * P:(st + 1) * P, :],
                                      hnbS[:, st, :])

            # ================= layers =================
            def emit_prep(w, stream, gt):
                wt = WT_L if stream == "L" else WT_H
                tg = t_gidxL if stream == "L" else t_gidxH
                nidx = wt * P
                gidx = gpool.tile([P, nidx // 16], I16, tag=f"i{stream}",
                                  name=f"i{stream}t")
                nc.sync.dma_start(
                    gidx[:], tg[:, w * (nidx // 16):(w + 1) * (nidx // 16)])
                tbl = (t_hfull[0:B_SPLIT, :] if stream == "L"
                       else t_hfull[B_SPLIT:NCORES * NPU, :])
                qa, qb = wqueues(w)
                q = qa if stream == "L" else qb
                if PREP:
                    nc.gpsimd.dma_gather(
                        gt[:], tbl, gidx[:],
                        nidx, nidx, D, single_packet=False,
                        prepare_only=True, sem=dma_sems[q], queue_num=q)
                else:
                    nc.gpsimd.dma_gather(
                        gt[:], tbl, gidx[:],
                        nidx, nidx, D, single_packet=False, queue_num=q)
                return q

            def new_gt(stream):
                wt = WT_L if stream == "L" else WT_H
                return gpool.tile([P, wt, D], HDT, tag=f"g{stream}",
                                  name=f"g{stream}t")

            def new_sel(w, stream):
                wt = WT_L if stream == "L" else WT_H
                tsel = t_selL if stream == "L" else t_selH
                sel = selpool_p.tile([P, wt, P], HDT, tag=f"s{stream}",
                                     name=f"s{stream}t")
                nc.sync.dma_start(
                    sel[:], tsel[:, w * (wt * P):(w + 1) * (wt * P)])
                return sel

            def new_ctk(w):
                ctk = wk.tile([P, NKC, WSZ], HDT, tag="ct", bufs=2)
                nc.sync.dma_start(
                    ctk[:], t_countT.ap().rearrange(
                        "(k p) n -> p k n", p=P)[:, :, w * WSZ:(w + 1) * WSZ])
                return ctk

            for l in range(L):
                # prologue: prep the first windows' gathers so descriptor gen
                # runs during the previous layer's AR/apply and this AG.
                # Only the triggers carry the h_full dependency.
                pre_gt = {}
                pre_sel = {}
                pre_ctk = {}
                if PREP:
                    for w in range(NPRE - 1):
                        for stream in ("L", "H"):
                            gt = new_gt(stream)
                            emit_prep(w, stream, gt)
                            pre_gt[(w, stream)] = gt
                allgather([t_hnew[:]], [t_hfull[:]])
                if PREP:
                    for stream in ("L", "H"):
                        gt = new_gt(stream)
                        emit_prep(NPRE - 1, stream, gt)
                        pre_gt[(NPRE - 1, stream)] = gt
                    for q in range(NQ):
                        nc.gpsimd.trigger_dma(count=None, queue_num=q)
                for w in range(NPRE - 1):
                    pre_ctk[w] = new_ctk(w)
                    for stream in ("L", "H"):
                        pre_sel[(w, stream)] = new_sel(w, stream)
                stats0 = sps.tile([1, D], F32, tag="stats0")
                stats1 = sps.tile([1, D], F32, tag="stats1")
                if l == L - 1:
                    poolps = [sps.tile([P, GPC], F32, tag=f"pool{h}",
                                       name=f"pool{h}") for h in range(KD)]
                for w in range(NWIN):
                    winp = [wps.tile([P, WSZ], F32, tag="win", name=f"win{h}")
                            for h in range(KD)]
                    ctk = pre_ctk.pop(w) if w in pre_ctk else new_ctk(w)
                    gts = {}
                    sels = {}
                    for stream in ("L", "H"):
                        if (w, stream) in pre_gt:
                            gts[stream] = pre_gt.pop((w, stream))
                        else:
                            gt = new_gt(stream)
                            q = emit_prep(w, stream, gt)
                            if PREP:
                                nc.gpsimd.trigger_dma(count=None, queue_num=q)
                            gts[stream] = gt
                        if (w, stream) in pre_sel:
                            sels[stream] = pre_sel.pop((w, stream))
                        else:
                            sels[stream] = new_sel(w, stream)
                    # bond term: window-wide, starts the PSUM accumulation
                    for kk in range(NKC):
                        for h in range(KD):
                            nc.tensor.matmul(
                                out=winp[h][:],
                                lhsT=t5S[:, l, kk, h * P:(h + 1) * P],
                                rhs=ctk[:, kk, :],
                                start=(kk == 0), stop=False)
                    for sw in range(SPW):
                        st = w * SPW + sw
                        # self term via identity (hnbS holds this layer's input)
                        for h in range(KD):
                            nc.tensor.matmul(
                                out=winp[h][:, sw * P:(sw + 1) * P],
                                lhsT=hnbS[:, st, h * P:(h + 1) * P],
                                rhs=identS[:],
                                start=False, stop=False)
                        for stream, tt in (("L", TT_L), ("H", TT_H)):
                            gt = gts[stream]
                            sel = sels[stream]
                            last_stream = stream == "H"
                            for t in range(tt):
                                ti = sw * tt + t
                                for h in range(KD):
                                    nc.tensor.matmul(
                                        out=winp[h][:, sw * P:(sw + 1) * P],
                                        lhsT=gt[:, ti, h * P:(h + 1) * P],
                                        rhs=sel[:, ti, :],
                                        start=False,
                                        stop=(last_stream and t == tt - 1))
                    # x^T = deginv * window  (bf16)
                    xt = [xtp.tile([P, WSZ], BF16, tag="xt", name=f"xt{h}")
                          for h in range(KD)]
                    for h in range(KD):
                        nc.vector.tensor_tensor(
                            out=xt[h][:], in0=winp[h][:],
                            in1=dgS[:, w * WSZ:(w + 1) * WSZ],
                            op=mybir.AluOpType.mult)
                    # update matmul + stats per subtile
                    for sw in range(SPW):
                        st = w * SPW + sw
                        hlp = hps.tile([P, D], F32, tag="hl")
                        for h in range(KD):
                            nc.tensor.matmul(
                                out=hlp[:],
                                lhsT=xt[h][:, sw * P:(sw + 1) * P],
                                rhs=wS[:, l, h, :],
                                start=(h == 0), stop=(h == KD - 1))
                        nc.scalar.activation(hlinS[:, st, :], hlp[:],
                                             mybir.ActivationFunctionType.Copy)
                        sq = wk.tile([P, D], BF16, tag="sq")
                        nc.vector.tensor_tensor(out=sq[:], in0=hlinS[:, st, :],
                                                in1=hlinS[:, st, :],
                                                op=mybir.AluOpType.mult)
                        nc.tensor.matmul(out=stats0[:],
                                         lhsT=onecol[:],
                                         rhs=hlinS[:, st, :],
                                         start=(st == 0), stop=(st == NSW - 1))
                        nc.tensor.matmul(out=stats1[:],
                                         lhsT=onecol[:], rhs=sq[:],
                                         start=(st == 0), stop=(st == NSW - 1))
                # --- BN stats allreduce + scale/shift ---
                stsb0 = smp.tile([1, D], F32, tag="stsb0")
                stsb1 = smp.tile([1, D], F32, tag="stsb1")
                nc.scalar.activation(stsb0[:], stats0[:],
                                     mybir.ActivationFunctionType.Copy)
                nc.scalar.activation(stsb1[:], stats1[:],
                                     mybir.ActivationFunctionType.Copy)
                nc.sync.dma_start(t_arin[l][0:1, :], stsb0[:])
                nc.sync.dma_start(t_arin[l][1:2, :], stsb1[:])
                allreduce([t_arin[l][:]], [t_arout[l][:]])
                stg0 = smp.tile([1, D], F32, tag="stg0")
                stg1 = smp.tile([1, D], F32, tag="stg1")
                nc.sync.dma_start(stg0[:], t_arout[l][0:1, :])
                nc.sync.dma_start(stg1[:], t_arout[l][1:2, :])
                mean = smp.tile([1, D], F32, tag="mean")
                nc.vector.tensor_scalar_mul(mean[:], stg0[:], 1.0 / NREAL)
                msq = smp.tile([1, D], F32, tag="msq")
                nc.vector.tensor_scalar_mul(msq[:], stg1[:], 1.0 / NREAL)
                var = smp.tile([1, D], F32, tag="var")
                nc.vector.tensor_tensor(out=var[:], in0=mean[:], in1=mean[:],
                                        op=mybir.AluOpType.mult)
                nc.vector.tensor_tensor(out=var[:], in0=msq[:], in1=var[:],
                                        op=mybir.AluOpType.subtract)
                sd = smp.tile([1, D], F32, tag="sd")
                nc.scalar.activation(sd[:], var[:],
                                     mybir.ActivationFunctionType.Sqrt,
                                     bias=epsS[:])
                rsq = smp.tile([1, D], F32, tag="rsq")
                nc.vector.reciprocal(rsq[:], sd[:])
                scl = smp.tile([1, D], F32, tag="scl")
                nc.vector.tensor_tensor(out=scl[:], in0=rsq[:],
                                        in1=gamS[:, l, :],
                                        op=mybir.AluOpType.mult)
                sft = smp.tile([1, D], F32, tag="sft")
                nc.vector.tensor_tensor(out=sft[:], in0=mean[:], in1=scl[:],
                                        op=mybir.AluOpType.mult)
                nc.vector.tensor_tensor(out=sft[:], in0=betS[:, l, :],
                                        in1=sft[:],
                                        op=mybir.AluOpType.subtract)
                ab = aps.tile([P, 2 * D], F32, tag="ab")
                nc.tensor.matmul(out=ab[:, 0:D], lhsT=onesS[:], rhs=scl[:],
                                 start=True, stop=True)
                nc.tensor.matmul(out=ab[:, D:2 * D], lhsT=onesS[:], rhs=sft[:],
                                 start=True, stop=True)
                # --- apply + (layer L-1) pooling ---
                for st in range(NSW):
                    hnf = wk.tile([P, D], F32, tag="hnf")
                    nc.vector.tensor_tensor(out=hnf[:], in0=hlinS[:, st, :],
                                            in1=ab[:, 0:D],
                                            op=mybir.AluOpType.mult)
                    nc.vector.tensor_tensor(out=hnf[:], in0=hnf[:],
                                            in1=ab[:, D:2 * D],
                                            op=mybir.AluOpType.add)
                    if l < L - 1:
                        nc.scalar.activation(hnbS[:, st, :], hnf[:],
                                             mybir.ActivationFunctionType.Relu)
                        nc.sync.dma_start(t_hnew[st * P:(st + 1) * P, :],
                                          hnbS[:, st, :])
                    else:
                        hnr = wk.tile([P, D], F32, tag="hnr")
                        nc.vector.tensor_scalar_max(hnr[:], hnf[:], 0.0)
                        for h in range(KD):
                            nc.tensor.matmul(
                                out=poolps[h][:],
                                lhsT=hnr[:, h * P:(h + 1) * P],
                                rhs=selpS[:, st, :],
                                start=(st == 0), stop=(st == NSW - 1))
                if l < L - 1:
                    allgather([t_hnew[:]], [t_hfull[:]])

            # ================= readout =================
            gts = smp.tile([P, KD * GPC], F32, tag="gts")
            for h in range(KD):
                nc.scalar.activation(gts[:, h * GPC:(h + 1) * GPC],
                                     poolps[h][:],
                                     mybir.ActivationFunctionType.Copy)
            ones16 = smp.tile([1, GPC], F32, tag="o16")
            nc.vector.memset(ones16[:], 1.0)
            outp = sps.tile([GPC, OUT], F32, tag="stats0")
            for h in range(KD):
                nc.tensor.matmul(out=outp[:],
                                 lhsT=gts[:, h * GPC:(h + 1) * GPC],
                                 rhs=wpS[:, h, :], start=(h == 0), stop=False)
            nc.tensor.matmul(out=outp[:], lhsT=ones16[:], rhs=bpS[:],
                             start=False, stop=True)
            outs = smp.tile([GPC, OUT], F32, tag="outs")
            nc.scalar.activation(outs[:], outp[:],
                                 mybir.ActivationFunctionType.Copy)
            nc.sync.dma_start(t_out[:], outs[:])

    nc.compile()
    return nc


LAST = {}


def kernel(**inputs):
    cfg, in_maps, _ = preprocess(inputs)
    nc = build(cfg)
    trace = os.environ.get("KGCN_TRACE") == "1"
    res = run_bass_kernel_spmd(nc, in_maps, list(range(NCORES)), trace=trace)
    LAST["exec_time_ns"] = res.exec_time_ns
    LAST["profile_json"] = res.profile_json
    out = np.concatenate([res.results[c]["out_g"] for c in range(NCORES)], 0)
    return out.astype(np.float32)


if __name__ == "__main__":
    pass


# revision 30
# speedup vs baseline: 1.3400x; 1.1128x over previous
"""GCN message-passing kernel for 8 Trainium2 NeuronCores (Bass/Tile).

v2 redesign vs v1 (4.66ms -> target <2.5ms):
- fp8 h table + Shared collective outputs by default (v1 had them off).
- Gathers merged per (window, stream): 26 DMAGatherAnt/layer instead of 104
  (amortizes ~1us fixed Q7 descriptor-gen cost per instruction).
- Selection matrices precomputed on host and DMA'd as inputs (removes the
  DVE IS_EQ chain, ~700us, plus GpSimd SBUF-port contention).
- Self-loop h term no longer gathered: post-activation h kept in SBUF
  (hnbS) and added to the PSUM window via identity matmuls (-5% descriptors).
- Bond-encoder matmuls window-wide (512-free) instead of per-subwindow.
- Dead-row zeroing dropped (pad edges have all-zero sel columns; pad nodes
  have deginv=0 and zero selpool rows, so garbage never propagates).
- AllReduce output Shared.
"""
import sys

sys.path.insert(0, "/opt/trn_rl_repo")

import os

import numpy as np
import ml_dtypes

import concourse.bass as bass
import concourse.bacc as bacc
import concourse.mybir as mybir
import concourse.tile as tile
from concourse.bass_utils import run_bass_kernel_spmd

P = 128
WSZ = 512          # psum node window
EPS = 1e-5
NCORES = 8
BF16 = mybir.dt.bfloat16
FP8 = mybir.dt.float8e4
F32 = mybir.dt.float32
I16 = mybir.dt.int16
S0 = 64.0          # layer-0 table scale (absorbed by BN)

USE_FP8 = os.environ.get("KGCN_FP8", "1") == "1"
HDT = FP8 if USE_FP8 else BF16
HNP = ml_dtypes.float8_e4m3fn if USE_FP8 else ml_dtypes.bfloat16


# ----------------------------------------------------------------------------
# Host preprocessing
# ----------------------------------------------------------------------------

def _wrap_idx(flat):
    n = flat.shape[0]
    assert n % 16 == 0
    w = flat.reshape(n // 16, 16).T.astype(np.int16)  # [16, n/16]
    return np.tile(w, (8, 1))


def preprocess(inputs, n_graphs=128):
    nfeat = np.asarray(inputs["nfeat"], np.int64)
    efeat = np.asarray(inputs["efeat"], np.int64)
    src = np.asarray(inputs["src"], np.int64)
    dst = np.asarray(inputs["dst"], np.int64)
    graph_ids = np.asarray(inputs["graph_ids"], np.int64)
    atom_emb = np.asarray(inputs["atom_emb"], np.float32)
    edge_emb = np.asarray(inputs["edge_emb"], np.float32)
    W = np.asarray(inputs["W"], np.float32)
    gamma = np.asarray(inputs["gamma"], np.float32)
    beta = np.asarray(inputs["beta"], np.float32)
    Wp = np.asarray(inputs["Wp"], np.float32)
    bp = np.asarray(inputs["bp"], np.float32)

    N = graph_ids.shape[0]
    E = src.shape[0]
    G = n_graphs
    GPC = G // NCORES
    AC, AV, D = atom_emb.shape
    L, BC, BV, _ = edge_emb.shape
    NCOMB = BV ** BC
    OUT = Wp.shape[1]
    HALF = NCORES // 2

    gcnt = np.bincount(graph_ids, minlength=G)
    gofs = np.concatenate([[0], np.cumsum(gcnt)])
    S = gofs[::GPC].astype(np.int64)
    assert S[-1] == N
    Nc = np.diff(S)

    NSW = int(np.ceil((Nc.max() + 1) / P))
    NPU = NSW * P
    NWIN = NPU // WSZ
    if NWIN * WSZ < NPU:
        NWIN += 1
        NPU = NWIN * WSZ
        NSW = NPU // P
    SPW = WSZ // P
    B_SPLIT = HALF * NPU
    assert B_SPLIT < 32768 and (NCORES - HALF) * NPU < 32768

    degs = np.bincount(dst, minlength=N).astype(np.float64) + 1.0
    deginv_all = (1.0 / degs).astype(np.float32)

    node_core = np.searchsorted(S[1:], np.arange(N), side="right").astype(np.int64)
    src_core = node_core[src]
    e_isL = src_core < HALF
    dLn = np.bincount(dst[e_isL], minlength=N)
    dHn = np.bincount(dst[~e_isL], minlength=N)

    # --- per-core node permutation: balance (dL, dH) across NSW bins ---
    pos_of_node = np.full(N, -1, np.int64)
    node_at_pos = [np.full(NPU, -1, np.int64) for _ in range(NCORES)]
    for c in range(NCORES):
        nodes = np.arange(S[c], S[c + 1])
        wl = dLn[nodes].astype(np.int64)
        wh = dHn[nodes].astype(np.int64)
        order = np.argsort(-(wl + wh), kind="stable")
        binL = np.zeros(NSW, np.int64)
        binH = np.zeros(NSW, np.int64)
        binN = np.zeros(NSW, np.int64)
        for i in order:
            nl, nh = wl[i], wh[i]
            cand = np.maximum(binL + nl, binH + nh) + 1e-3 * (binL + binH)
            cand[binN >= P] = 1 << 60
            b = int(np.argmin(cand))
            slot = binN[b]
            binN[b] += 1
            binL[b] += nl
            binH[b] += nh
            n = nodes[i]
            pos_of_node[n] = c * NPU + b * P + slot
            node_at_pos[c][b * P + slot] = n

    # --- edge streams per (core, stream): edges only, sorted by dst ---
    src_pg = pos_of_node[src]
    dst_pos = pos_of_node[dst]
    dst_core = node_core[dst]

    core_streams = []   # [core][stream] -> (srcpos_sorted, dstlocal_sorted)
    for c in range(NCORES):
        em = dst_core == c
        es, ed, eL = src_pg[em], dst_pos[em] - c * NPU, e_isL[em]
        per = {}
        for stream, m in (("L", eL), ("H", ~eL)):
            ssrc = es[m]
            sdst = ed[m]
            o = np.argsort(sdst, kind="stable")
            ssrc, sdst = ssrc[o], sdst[o]
            if stream == "H":
                ssrc = ssrc - B_SPLIT
            per[stream] = (ssrc, sdst)
        core_streams.append(per)

    # tiles per (sw, stream) = global max of ceil(edges_sw / P)
    TT = {}
    for stream in ("L", "H"):
        mx = 1
        for c in range(NCORES):
            _, sdst = core_streams[c][stream]
            cnt = np.bincount(sdst // P, minlength=NSW)
            mx = max(mx, int(np.ceil(cnt.max() / P)))
        TT[stream] = mx

    def pack_core(c):
        out = {}
        for stream in ("L", "H"):
            ssrc, sdst = core_streams[c][stream]
            tt = TT[stream]
            idx = np.zeros((NSW, tt * P), np.int16)   # pad -> row 0 (sel=0)
            sel = np.zeros((P, NSW * tt, P), np.float32)
            sw_of = sdst // P
            starts = np.concatenate([[0], np.cumsum(np.bincount(sw_of, minlength=NSW))])
            for sw in range(NSW):
                r0, r1 = starts[sw], starts[sw + 1]
                k = r1 - r0
                assert k <= tt * P
                idx[sw, :k] = ssrc[r0:r1]
                rows = np.arange(k)
                sel[rows % P, sw * tt + rows // P, sdst[r0:r1] - sw * P] = 1.0
            out[stream] = (idx.reshape(-1), sel.astype(HNP))
        return out

    packed = [pack_core(c) for c in range(NCORES)]

    # --- tables ---
    # atom9[p, a, :] = atom_emb[a, p, :] * S0  (vocab entry p of column a)
    atom9_q = (np.transpose(atom_emb, (1, 0, 2)) * S0).astype(HNP)

    k = np.arange(NCOMB)
    d0, d1, d2 = k // (BV * BV), (k // BV) % BV, k % BV
    T512 = edge_emb[:, 0, d0] + edge_emb[:, 1, d1] + edge_emb[:, 2, d2]
    T512[0] *= S0
    T512_q = T512.astype(HNP)

    cidx = (efeat[:, 0] * BV + efeat[:, 1]) * BV + efeat[:, 2]

    cfg = dict(N=N, E=E, G=G, GPC=GPC, D=D, L=L, OUT=OUT, NPU=NPU, NSW=NSW,
               NWIN=NWIN, SPW=SPW, TT_L=TT["L"], TT_H=TT["H"],
               B_SPLIT=B_SPLIT, NCOMB=NCOMB, AC=AC, NREAL=N)

    in_maps = []
    for c in range(NCORES):
        m = {}
        for stream in ("L", "H"):
            idx, sel = packed[c][stream]
            m[f"gidx{stream}"] = _wrap_idx(idx)
            m[f"sel{stream}"] = sel.reshape(P, -1).copy()
        em = dst_core == c
        lp = dst_pos[em] - c * NPU
        ct = np.zeros((NCOMB, NPU), np.float32)
        np.add.at(ct, (cidx[em], lp), 1.0)
        m["countT"] = ct.astype(HNP)
        dg = np.zeros(NPU, np.float32)
        rp = node_at_pos[c] >= 0
        dg[rp] = deginv_all[node_at_pos[c][rp]]
        m["deginv"] = np.tile(dg[None, :], (P, 1)).astype(ml_dtypes.bfloat16)
        sp = np.zeros((NPU, GPC), np.float32)
        gl = np.where(rp)[0]
        gid = graph_ids[node_at_pos[c][gl]] - c * GPC
        cnts = np.maximum(gcnt[c * GPC:(c + 1) * GPC], 1.0)
        sp[gl, gid] = (1.0 / cnts[gid]).astype(np.float32)
        m["selpool"] = sp
        cnt9 = np.zeros((P, AC, NPU), HNP)
        pos_r = np.where(rp)[0]
        nf = nfeat[node_at_pos[c][pos_r]]
        for a in range(AC):
            cnt9[nf[:, a], a, pos_r] = 1.0
        m["cnt9"] = cnt9.reshape(P, -1).copy()
        m["atom9"] = atom9_q
        m["t512"] = T512_q
        m["wl"] = W.astype(ml_dtypes.bfloat16)             # [L, D, D]
        m["gam"] = gamma.reshape(L, 1, D).copy()
        m["bet"] = beta.reshape(L, 1, D).copy()
        m["wp"] = Wp.copy()
        m["bpr"] = bp.reshape(1, OUT).copy()
        m["ident"] = np.eye(P, dtype=HNP)
        in_maps.append(m)

    meta = dict(S=S, Nc=Nc)
    return cfg, in_maps, meta


# ----------------------------------------------------------------------------
# Device kernel builder (uniform SPMD program)
# ----------------------------------------------------------------------------

def build(cfg):
    D = cfg["D"]; L = cfg["L"]; NPU = cfg["NPU"]; NSW = cfg["NSW"]
    NWIN = cfg["NWIN"]; SPW = cfg["SPW"]; TT_L = cfg["TT_L"]; TT_H = cfg["TT_H"]
    NCOMB = cfg["NCOMB"]; AC = cfg["AC"]; GPC = cfg["GPC"]; OUT = cfg["OUT"]
    B_SPLIT = cfg["B_SPLIT"]; NREAL = cfg["NREAL"]
    KD = D // P
    NKC = NCOMB // P
    NT_L, NT_H = NSW * TT_L, NSW * TT_H
    WT_L, WT_H = SPW * TT_L, SPW * TT_H     # gather tiles per window
    NQ = int(os.environ.get("KGCN_NQ", "4"))
    PREP = os.environ.get("KGCN_PREP", "0") == "1"
    NPRE = 3                                 # windows prepped ahead at layer start

    nc = bacc.Bacc("TRN2", target_bir_lowering=False, debug=False,
                   num_devices=NCORES, num_swdge_queues=NQ)

    def allgather(ins, outs):
        if USE_FP8:
            ins = [ap.bitcast(BF16) for ap in ins]
            outs = [ap.bitcast(BF16) for ap in outs]
        nc.gpsimd.collective_compute(
            "AllGather", mybir.AluOpType.bypass,
            replica_groups=[list(range(NCORES))], ins=ins, outs=outs)

    def allgather_f32(ins, outs):
        nc.gpsimd.collective_compute(
            "AllGather", mybir.AluOpType.bypass,
            replica_groups=[list(range(NCORES))], ins=ins, outs=outs)

    t_gidxL = nc.dram_tensor("gidxL", [P, NT_L * P // 16], I16, kind="ExternalInput")
    t_gidxH = nc.dram_tensor("gidxH", [P, NT_H * P // 16], I16, kind="ExternalInput")
    t_selL = nc.dram_tensor("selL", [P, NT_L * P], HDT, kind="ExternalInput")
    t_selH = nc.dram_tensor("selH", [P, NT_H * P], HDT, kind="ExternalInput")
    t_countT = nc.dram_tensor("countT", [NCOMB, NPU], HDT, kind="ExternalInput")
    t_deginv = nc.dram_tensor("deginv", [P, NPU], BF16, kind="ExternalInput")
    t_selpool = nc.dram_tensor("selpool", [NPU, GPC], F32, kind="ExternalInput")
    t_cnt9 = nc.dram_tensor("cnt9", [P, AC * NPU], HDT, kind="ExternalInput")
    t_atom9 = nc.dram_tensor("atom9", [P, AC, D], HDT, kind="ExternalInput")
    t_t512 = nc.dram_tensor("t512", [L, NCOMB, D], HDT, kind="ExternalInput")
    t_wl = nc.dram_tensor("wl", [L, D, D], BF16, kind="ExternalInput")
    t_gam = nc.dram_tensor("gam", [L, 1, D], F32, kind="ExternalInput")
    t_bet = nc.dram_tensor("bet", [L, 1, D], F32, kind="ExternalInput")
    t_wp = nc.dram_tensor("wp", [D, OUT], F32, kind="ExternalInput")
    t_bp = nc.dram_tensor("bpr", [1, OUT], F32, kind="ExternalInput")
    t_ident = nc.dram_tensor("ident", [P, P], HDT, kind="ExternalInput")
    t_out = nc.dram_tensor("out_g", [GPC, OUT], F32, kind="ExternalOutput")
    t_hfull = nc.dram_tensor("h_full", [NCORES * NPU, D], HDT, addr_space="Shared")
    t_hnew = nc.dram_tensor("h_newc", [NPU, D], HDT)
    t_arin = [nc.dram_tensor(f"arin{l}", [1, 2 * D], F32) for l in range(L)]
    t_arout = [nc.dram_tensor(f"arout{l}", [NCORES, 2 * D], F32,
                              addr_space="Shared") for l in range(L)]

    dma_sems = [nc.alloc_semaphore(f"swdge_dma{q}") for q in range(NQ)]

    def wqueues(w):
        qa = (2 * w) % NQ
        return qa, qa + 1

    with tile.TileContext(nc) as tc:
        with (
            tc.tile_pool(name="static", bufs=1) as stp,
            tc.tile_pool(name="gath", bufs=3) as gpool,
            tc.tile_pool(name="selp", bufs=2) as selpool_p,
            tc.tile_pool(name="xt", bufs=2) as xtp,
            tc.tile_pool(name="work", bufs=3) as wk,
            tc.tile_pool(name="small", bufs=1) as smp,
            tc.tile_pool(name="winps", bufs=2, space="PSUM") as wps,
            tc.tile_pool(name="hlps", bufs=1, space="PSUM") as hps,
            tc.tile_pool(name="smps", bufs=1, space="PSUM") as sps,
            tc.tile_pool(name="abps", bufs=1, space="PSUM") as aps,
        ):
            # ---- static SBUF preloads ----
            atom9S = stp.tile([P, AC, D], HDT)
            selpS = stp.tile([P, NSW, GPC], F32)
            dgS = stp.tile([P, NPU], BF16)
            wS = stp.tile([P, L, KD, D], BF16)
            t5S = stp.tile([P, L, NKC, D], HDT)
            gamS = stp.tile([1, L, D], F32)
            betS = stp.tile([1, L, D], F32)
            wpS = stp.tile([P, KD, OUT], F32)
            bpS = stp.tile([1, OUT], F32)
            onesS = stp.tile([1, P], F32)
            onecol = stp.tile([P, 1], BF16)
            identS = stp.tile([P, P], HDT)
            hlinS = stp.tile([P, NSW, D], BF16)
            hnbS = stp.tile([P, NSW, D], HDT)
            epsS = stp.tile([1, 1], F32)
            nc.vector.memset(epsS[:], EPS)
            nc.sync.dma_start(atom9S[:], t_atom9[:])
            nc.sync.dma_start(identS[:], t_ident[:])
            nc.sync.dma_start(selpS[:], t_selpool.ap().rearrange("(s p) g -> p s g", p=P))
            nc.sync.dma_start(dgS[:], t_deginv[:])
            nc.sync.dma_start(wS[:], t_wl.ap().rearrange("l (k p) d -> p l k d", p=P))
            nc.sync.dma_start(t5S[:], t_t512.ap().rearrange("l (k p) d -> p l k d", p=P))
            nc.sync.dma_start(gamS[:], t_gam.ap().rearrange("l o d -> o l d"))
            nc.sync.dma_start(betS[:], t_bet.ap().rearrange("l o d -> o l d"))
            nc.sync.dma_start(wpS[:], t_wp.ap().rearrange("(k p) o -> p k o", p=P))
            nc.sync.dma_start(bpS[:], t_bp[:])
            nc.vector.memset(onesS[:], 1.0)
            nc.vector.memset(onecol[:], 1.0)
            ones8 = stp.tile([NCORES, 1], F32)
            nc.vector.memset(ones8[:], 1.0)

            # ============ h0: atom embedding sums via count matmuls ============
            for w in range(NWIN):
                cnt = wk.tile([P, AC, WSZ], HDT, tag="cnt", bufs=2)
                nc.sync.dma_start(
                    cnt[:], t_cnt9.ap().rearrange("p (a n) -> p a n", a=AC)
                    [:, :, w * WSZ:(w + 1) * WSZ])
                for sw in range(SPW):
                    st = w * SPW + sw
                    h0p = hps.tile([P, D], F32, tag="hl")
                    for a in range(AC):
                        nc.tensor.matmul(
                            out=h0p[:],
                            lhsT=cnt[:, a, sw * P:(sw + 1) * P],
                            rhs=atom9S[:, a, :],
                            start=(a == 0), stop=(a == AC - 1))
                    nc.scalar.activation(hnbS[:, st, :], h0p[:],
                                         mybir.ActivationFunctionType.Copy)
                    nc.sync.dma_start(t_hnew[st * P:(st + 1) * P, :],
                                      hnbS[:, st, :])

            # ================= layers =================
            def emit_prep(w, stream, gt):
                wt = WT_L if stream == "L" else WT_H
                tg = t_gidxL if stream == "L" else t_gidxH
                nidx = wt * P
                gidx = gpool.tile([P, nidx // 16], I16, tag=f"i{stream}",
                                  name=f"i{stream}t")
                nc.sync.dma_start(
                    gidx[:], tg[:, w * (nidx // 16):(w + 1) * (nidx // 16)])
                tbl = (t_hfull[0:B_SPLIT, :] if stream == "L"
                       else t_hfull[B_SPLIT:NCORES * NPU, :])
                qa, qb = wqueues(w)
                q = qa if stream == "L" else qb
                if PREP:
                    nc.gpsimd.dma_gather(
                        gt[:], tbl, gidx[:],
                        nidx, nidx, D, single_packet=False,
                        prepare_only=True, sem=dma_sems[q], queue_num=q)
                else:
                    nc.gpsimd.dma_gather(
                        gt[:], tbl, gidx[:],
                        nidx, nidx, D, single_packet=False, queue_num=q)
                return q

            def new_gt(stream):
                wt = WT_L if stream == "L" else WT_H
                return gpool.tile([P, wt, D], HDT, tag=f"g{stream}",
                                  name=f"g{stream}t")

            def new_sel(w, stream):
                wt = WT_L if stream == "L" else WT_H
                tsel = t_selL if stream == "L" else t_selH
                sel = selpool_p.tile([P, wt, P], HDT, tag=f"s{stream}",
                                     name=f"s{stream}t")
                nc.sync.dma_start(
                    sel[:], tsel[:, w * (wt * P):(w + 1) * (wt * P)])
                return sel

            def new_ctk(w):
                ctk = wk.tile([P, NKC, WSZ], HDT, tag="ct", bufs=2)
                nc.sync.dma_start(
                    ctk[:], t_countT.ap().rearrange(
                        "(k p) n -> p k n", p=P)[:, :, w * WSZ:(w + 1) * WSZ])
                return ctk

            for l in range(L):
                # prologue: prep the first windows' gathers so descriptor gen
                # runs during the previous layer's AR/apply and this AG.
                # Only the triggers carry the h_full dependency.
                pre_gt = {}
                pre_sel = {}
                pre_ctk = {}
                if PREP:
                    for w in range(NPRE - 1):
                        for stream in ("L", "H"):
                            gt = new_gt(stream)
                            emit_prep(w, stream, gt)
                            pre_gt[(w, stream)] = gt
                allgather([t_hnew[:]], [t_hfull[:]])
                if PREP:
                    for stream in ("L", "H"):
                        gt = new_gt(stream)
                        emit_prep(NPRE - 1, stream, gt)
                        pre_gt[(NPRE - 1, stream)] = gt
                    for q in range(NQ):
                        nc.gpsimd.trigger_dma(count=None, queue_num=q)
                for w in range(NPRE - 1):
                    pre_ctk[w] = new_ctk(w)
                    for stream in ("L", "H"):
                        pre_sel[(w, stream)] = new_sel(w, stream)
                stats0 = sps.tile([1, D], F32, tag="stats0")
                stats1 = sps.tile([1, D], F32, tag="stats1")
                if l == L - 1:
                    poolps = [sps.tile([P, GPC], F32, tag=f"pool{h}",
                                       name=f"pool{h}") for h in range(KD)]
                for w in range(NWIN):
                    winp = [wps.tile([P, WSZ], F32, tag="win", name=f"win{h}")
                            for h in range(KD)]
                    ctk = pre_ctk.pop(w) if w in pre_ctk else new_ctk(w)
                    gts = {}
                    sels = {}
                    for stream in ("L", "H"):
                        if (w, stream) in pre_gt:
                            gts[stream] = pre_gt.pop((w, stream))
                        else:
                            gt = new_gt(stream)
                            q = emit_prep(w, stream, gt)
                            if PREP:
                                nc.gpsimd.trigger_dma(count=None, queue_num=q)
                            gts[stream] = gt
                        if (w, stream) in pre_sel:
                            sels[stream] = pre_sel.pop((w, stream))
                        else:
                            sels[stream] = new_sel(w, stream)
                    # bond term: window-wide, starts the PSUM accumulation
                    for kk in range(NKC):
                        for h in range(KD):
                            nc.tensor.matmul(
                                out=winp[h][:],
                                lhsT=t5S[:, l, kk, h * P:(h + 1) * P],
                                rhs=ctk[:, kk, :],
                                start=(kk == 0), stop=False)
                    for sw in range(SPW):
                        st = w * SPW + sw
                        # self term via identity (hnbS holds this layer's input)
                        for h in range(KD):
                            nc.tensor.matmul(
                                out=winp[h][:, sw * P:(sw + 1) * P],
                                lhsT=hnbS[:, st, h * P:(h + 1) * P],
                                rhs=identS[:],
                                start=False, stop=False)
                        for stream, tt in (("L", TT_L), ("H", TT_H)):
                            gt = gts[stream]
                            sel = sels[stream]
                            last_stream = stream == "H"
                            for t in range(tt):
                                ti = sw * tt + t
                                for h in range(KD):
                                    nc.tensor.matmul(
                                        out=winp[h][:, sw * P:(sw + 1) * P],
                                        lhsT=gt[:, ti, h * P:(h + 1) * P],
                                        rhs=sel[:, ti, :],
                                        start=False,
                                        stop=(last_stream and t == tt - 1))
                    # x^T = deginv * window  (bf16)
                    xt = [xtp.tile([P, WSZ], BF16, tag="xt", name=f"xt{h}")
                          for h in range(KD)]
                    for h in range(KD):
                        nc.vector.tensor_tensor(
                            out=xt[h][:], in0=winp[h][:],
                            in1=dgS[:, w * WSZ:(w + 1) * WSZ],
                            op=mybir.AluOpType.mult)
                    # update matmul + stats per subtile
                    for sw in range(SPW):
                        st = w * SPW + sw
                        hlp = hps.tile([P, D], F32, tag="hl")
                        for h in range(KD):
                            nc.tensor.matmul(
                                out=hlp[:],
                                lhsT=xt[h][:, sw * P:(sw + 1) * P],
                                rhs=wS[:, l, h, :],
                                start=(h == 0), stop=(h == KD - 1))
                        nc.scalar.activation(hlinS[:, st, :], hlp[:],
                                             mybir.ActivationFunctionType.Copy)
                        sq = wk.tile([P, D], BF16, tag="sq")
                        nc.vector.tensor_tensor(out=sq[:], in0=hlinS[:, st, :],
                                                in1=hlinS[:, st, :],
                                                op=mybir.AluOpType.mult)
                        nc.tensor.matmul(out=stats0[:],
                                         lhsT=onecol[:],
                                         rhs=hlinS[:, st, :],
                                         start=(st == 0), stop=(st == NSW - 1))
                        nc.tensor.matmul(out=stats1[:],
                                         lhsT=onecol[:], rhs=sq[:],
                                         start=(st == 0), stop=(st == NSW - 1))
                # --- BN stats: AllGather per-core stats, reduce locally ---
                stsb = smp.tile([1, 2 * D], F32, tag="stsb")
                nc.scalar.activation(stsb[:, 0:D], stats0[:],
                                     mybir.ActivationFunctionType.Copy)
                nc.scalar.activation(stsb[:, D:2 * D], stats1[:],
                                     mybir.ActivationFunctionType.Copy)
                nc.sync.dma_start(t_arin[l][:], stsb[:])
                allgather_f32([t_arin[l][:]], [t_arout[l][:]])
                stg = smp.tile([NCORES, 2 * D], F32, tag="stg")
                nc.sync.dma_start(stg[:], t_arout[l][:])
                stsum = aps.tile([1, 2 * D], F32, tag="stsum")
                nc.tensor.matmul(out=stsum[:], lhsT=ones8[:], rhs=stg[:],
                                 start=True, stop=True)
                mean = smp.tile([1, D], F32, tag="mean")
                nc.vector.tensor_scalar_mul(mean[:], stsum[:, 0:D], 1.0 / NREAL)
                msq = smp.tile([1, D], F32, tag="msq")
                nc.vector.tensor_scalar_mul(msq[:], stsum[:, D:2 * D],
                                            1.0 / NREAL)
                var = smp.tile([1, D], F32, tag="var")
                nc.vector.tensor_tensor(out=var[:], in0=mean[:], in1=mean[:],
                                        op=mybir.AluOpType.mult)
                nc.vector.tensor_tensor(out=var[:], in0=msq[:], in1=var[:],
                                        op=mybir.AluOpType.subtract)
                sd = smp.tile([1, D], F32, tag="sd")
                nc.scalar.activation(sd[:], var[:],
                                     mybir.ActivationFunctionType.Sqrt,
                                     bias=epsS[:])
                rsq = smp.tile([1, D], F32, tag="rsq")
                nc.vector.reciprocal(rsq[:], sd[:])
                scl = smp.tile([1, D], F32, tag="scl")
                nc.vector.tensor_tensor(out=scl[:], in0=rsq[:],
                                        in1=gamS[:, l, :],
                                        op=mybir.AluOpType.mult)
                sft = smp.tile([1, D], F32, tag="sft")
                nc.vector.tensor_tensor(out=sft[:], in0=mean[:], in1=scl[:],
                                        op=mybir.AluOpType.mult)
                nc.vector.tensor_tensor(out=sft[:], in0=betS[:, l, :],
                                        in1=sft[:],
                                        op=mybir.AluOpType.subtract)
                ab = aps.tile([P, 2 * D], F32, tag="ab")
                nc.tensor.matmul(out=ab[:, 0:D], lhsT=onesS[:], rhs=scl[:],
                                 start=True, stop=True)
                nc.tensor.matmul(out=ab[:, D:2 * D], lhsT=onesS[:], rhs=sft[:],
                                 start=True, stop=True)
                # --- apply (window-batched) + (layer L-1) pooling ---
                a0 = ab[:, 0:D]
                a0b = bass.AP(a0.tensor, a0.offset,
                              [a0.ap[0], [0, SPW], a0.ap[1]])
                a1 = ab[:, D:2 * D]
                a1b = bass.AP(a1.tensor, a1.offset,
                              [a1.ap[0], [0, SPW], a1.ap[1]])
                for w in range(NWIN):
                    hnf = wk.tile([P, SPW, D], F32, tag="hnf", bufs=2)
                    nc.vector.tensor_tensor(
                        out=hnf[:], in0=hlinS[:, w * SPW:(w + 1) * SPW, :],
                        in1=a0b, op=mybir.AluOpType.mult)
                    nc.vector.tensor_tensor(out=hnf[:], in0=hnf[:],
                                            in1=a1b, op=mybir.AluOpType.add)
                    if l < L - 1:
                        nc.scalar.activation(
                            hnbS[:, w * SPW:(w + 1) * SPW, :], hnf[:],
                            mybir.ActivationFunctionType.Relu)
                        nc.sync.dma_start(
                            t_hnew.ap()[w * WSZ:(w + 1) * WSZ, :].rearrange(
                                "(s p) d -> p s d", p=P),
                            hnbS[:, w * SPW:(w + 1) * SPW, :])
                    else:
                        hnr = wk.tile([P, SPW, D], F32, tag="hnr", bufs=2)
                        nc.vector.tensor_scalar_max(hnr[:], hnf[:], 0.0)
                        for sw in range(SPW):
                            st = w * SPW + sw
                            for h in range(KD):
                                nc.tensor.matmul(
                                    out=poolps[h][:],
                                    lhsT=hnr[:, sw, h * P:(h + 1) * P],
                                    rhs=selpS[:, st, :],
                                    start=(st == 0), stop=(st == NSW - 1))

            # ================= readout =================
            gts = smp.tile([P, KD * GPC], F32, tag="gts")
            for h in range(KD):
                nc.scalar.activation(gts[:, h * GPC:(h + 1) * GPC],
                                     poolps[h][:],
                                     mybir.ActivationFunctionType.Copy)
            ones16 = smp.tile([1, GPC], F32, tag="o16")
            nc.vector.memset(ones16[:], 1.0)
            outp = sps.tile([GPC, OUT], F32, tag="stats0")
            for h in range(KD):
                nc.tensor.matmul(out=outp[:],
                                 lhsT=gts[:, h * GPC:(h + 1) * GPC],
                                 rhs=wpS[:, h, :], start=(h == 0), stop=False)
            nc.tensor.matmul(out=outp[:], lhsT=ones16[:], rhs=bpS[:],
                             start=False, stop=True)
            outs = smp.tile([GPC, OUT], F32, tag="outs")
            nc.scalar.activation(outs[:], outp[:],
                                 mybir.ActivationFunctionType.Copy)
            nc.sync.dma_start(t_out[:], outs[:])

    nc.compile()
    return nc


LAST = {}


def kernel(**inputs):
    cfg, in_maps, _ = preprocess(inputs)
    nc = build(cfg)
    trace = os.environ.get("KGCN_TRACE") == "1"
    res = run_bass_kernel_spmd(nc, in_maps, list(range(NCORES)), trace=trace)
    LAST["exec_time_ns"] = res.exec_time_ns
    LAST["profile_json"] = res.profile_json
    out = np.concatenate([res.results[c]["out_g"] for c in range(NCORES)], 0)
    return out.astype(np.float32)


if __name__ == "__main__":
    pass


# revision 31
# speedup vs baseline: 1.3601x; 1.0150x over previous
"""GCN message-passing kernel for 8 Trainium2 NeuronCores (Bass/Tile).

v2 redesign vs v1 (4.66ms -> target <2.5ms):
- fp8 h table + Shared collective outputs by default (v1 had them off).
- Gathers merged per (window, stream): 26 DMAGatherAnt/layer instead of 104
  (amortizes ~1us fixed Q7 descriptor-gen cost per instruction).
- Selection matrices precomputed on host and DMA'd as inputs (removes the
  DVE IS_EQ chain, ~700us, plus GpSimd SBUF-port contention).
- Self-loop h term no longer gathered: post-activation h kept in SBUF
  (hnbS) and added to the PSUM window via identity matmuls (-5% descriptors).
- Bond-encoder matmuls window-wide (512-free) instead of per-subwindow.
- Dead-row zeroing dropped (pad edges have all-zero sel columns; pad nodes
  have deginv=0 and zero selpool rows, so garbage never propagates).
- AllReduce output Shared.
"""
import sys

sys.path.insert(0, "/opt/trn_rl_repo")

import os

import numpy as np
import ml_dtypes

import concourse.bass as bass
import concourse.bacc as bacc
import concourse.mybir as mybir
import concourse.tile as tile
from concourse.bass_utils import run_bass_kernel_spmd

P = 128
WSZ = 512          # psum node window
EPS = 1e-5
NCORES = 8
BF16 = mybir.dt.bfloat16
FP8 = mybir.dt.float8e4
F32 = mybir.dt.float32
I16 = mybir.dt.int16
S0 = 64.0          # layer-0 table scale (absorbed by BN)

USE_FP8 = os.environ.get("KGCN_FP8", "1") == "1"
HDT = FP8 if USE_FP8 else BF16
HNP = ml_dtypes.float8_e4m3fn if USE_FP8 else ml_dtypes.bfloat16


# ----------------------------------------------------------------------------
# Host preprocessing
# ----------------------------------------------------------------------------

def _wrap_idx(flat):
    n = flat.shape[0]
    assert n % 16 == 0
    w = flat.reshape(n // 16, 16).T.astype(np.int16)  # [16, n/16]
    return np.tile(w, (8, 1))


def preprocess(inputs, n_graphs=128):
    nfeat = np.asarray(inputs["nfeat"], np.int64)
    efeat = np.asarray(inputs["efeat"], np.int64)
    src = np.asarray(inputs["src"], np.int64)
    dst = np.asarray(inputs["dst"], np.int64)
    graph_ids = np.asarray(inputs["graph_ids"], np.int64)
    atom_emb = np.asarray(inputs["atom_emb"], np.float32)
    edge_emb = np.asarray(inputs["edge_emb"], np.float32)
    W = np.asarray(inputs["W"], np.float32)
    gamma = np.asarray(inputs["gamma"], np.float32)
    beta = np.asarray(inputs["beta"], np.float32)
    Wp = np.asarray(inputs["Wp"], np.float32)
    bp = np.asarray(inputs["bp"], np.float32)

    N = graph_ids.shape[0]
    E = src.shape[0]
    G = n_graphs
    GPC = G // NCORES
    AC, AV, D = atom_emb.shape
    L, BC, BV, _ = edge_emb.shape
    NCOMB = BV ** BC
    OUT = Wp.shape[1]
    HALF = NCORES // 2

    gcnt = np.bincount(graph_ids, minlength=G)
    gofs = np.concatenate([[0], np.cumsum(gcnt)])
    S = gofs[::GPC].astype(np.int64)
    assert S[-1] == N
    Nc = np.diff(S)

    NSW = int(np.ceil((Nc.max() + 1) / P))
    NPU = NSW * P
    NWIN = NPU // WSZ
    if NWIN * WSZ < NPU:
        NWIN += 1
        NPU = NWIN * WSZ
        NSW = NPU // P
    SPW = WSZ // P
    B_SPLIT = HALF * NPU
    assert B_SPLIT < 32768 and (NCORES - HALF) * NPU < 32768

    degs = np.bincount(dst, minlength=N).astype(np.float64) + 1.0
    deginv_all = (1.0 / degs).astype(np.float32)

    node_core = np.searchsorted(S[1:], np.arange(N), side="right").astype(np.int64)
    src_core = node_core[src]
    e_isL = src_core < HALF
    dLn = np.bincount(dst[e_isL], minlength=N)
    dHn = np.bincount(dst[~e_isL], minlength=N)

    # --- per-core node permutation: balance (dL, dH) across NSW bins ---
    pos_of_node = np.full(N, -1, np.int64)
    node_at_pos = [np.full(NPU, -1, np.int64) for _ in range(NCORES)]
    for c in range(NCORES):
        nodes = np.arange(S[c], S[c + 1])
        wl = dLn[nodes].astype(np.int64)
        wh = dHn[nodes].astype(np.int64)
        order = np.argsort(-(wl + wh), kind="stable")
        binL = np.zeros(NSW, np.int64)
        binH = np.zeros(NSW, np.int64)
        binN = np.zeros(NSW, np.int64)
        for i in order:
            nl, nh = wl[i], wh[i]
            cand = np.maximum(binL + nl, binH + nh) + 1e-3 * (binL + binH)
            cand[binN >= P] = 1 << 60
            b = int(np.argmin(cand))
            slot = binN[b]
            binN[b] += 1
            binL[b] += nl
            binH[b] += nh
            n = nodes[i]
            pos_of_node[n] = c * NPU + b * P + slot
            node_at_pos[c][b * P + slot] = n

    # --- edge streams per (core, stream): edges only, sorted by dst ---
    src_pg = pos_of_node[src]
    dst_pos = pos_of_node[dst]
    dst_core = node_core[dst]

    core_streams = []   # [core][stream] -> (srcpos_sorted, dstlocal_sorted)
    for c in range(NCORES):
        em = dst_core == c
        es, ed, eL = src_pg[em], dst_pos[em] - c * NPU, e_isL[em]
        per = {}
        for stream, m in (("L", eL), ("H", ~eL)):
            ssrc = es[m]
            sdst = ed[m]
            o = np.argsort(sdst, kind="stable")
            ssrc, sdst = ssrc[o], sdst[o]
            if stream == "H":
                ssrc = ssrc - B_SPLIT
            per[stream] = (ssrc, sdst)
        core_streams.append(per)

    # tiles per (sw, stream) = global max of ceil(edges_sw / P)
    TT = {}
    for stream in ("L", "H"):
        mx = 1
        for c in range(NCORES):
            _, sdst = core_streams[c][stream]
            cnt = np.bincount(sdst // P, minlength=NSW)
            mx = max(mx, int(np.ceil(cnt.max() / P)))
        TT[stream] = mx

    def pack_core(c):
        out = {}
        for stream in ("L", "H"):
            ssrc, sdst = core_streams[c][stream]
            tt = TT[stream]
            idx = np.zeros((NSW, tt * P), np.int16)   # pad -> row 0 (sel=0)
            sel = np.zeros((P, NSW * tt, P), np.float32)
            sw_of = sdst // P
            starts = np.concatenate([[0], np.cumsum(np.bincount(sw_of, minlength=NSW))])
            for sw in range(NSW):
                r0, r1 = starts[sw], starts[sw + 1]
                k = r1 - r0
                assert k <= tt * P
                idx[sw, :k] = ssrc[r0:r1]
                rows = np.arange(k)
                sel[rows % P, sw * tt + rows // P, sdst[r0:r1] - sw * P] = 1.0
            out[stream] = (idx.reshape(-1), sel.astype(HNP))
        return out

    packed = [pack_core(c) for c in range(NCORES)]

    # --- tables ---
    # atom9[p, a, :] = atom_emb[a, p, :] * S0  (vocab entry p of column a)
    atom9_q = (np.transpose(atom_emb, (1, 0, 2)) * S0).astype(HNP)

    k = np.arange(NCOMB)
    d0, d1, d2 = k // (BV * BV), (k // BV) % BV, k % BV
    T512 = edge_emb[:, 0, d0] + edge_emb[:, 1, d1] + edge_emb[:, 2, d2]
    T512[0] *= S0
    T512_q = T512.astype(HNP)

    cidx = (efeat[:, 0] * BV + efeat[:, 1]) * BV + efeat[:, 2]

    cfg = dict(N=N, E=E, G=G, GPC=GPC, D=D, L=L, OUT=OUT, NPU=NPU, NSW=NSW,
               NWIN=NWIN, SPW=SPW, TT_L=TT["L"], TT_H=TT["H"],
               B_SPLIT=B_SPLIT, NCOMB=NCOMB, AC=AC, NREAL=N)

    in_maps = []
    for c in range(NCORES):
        m = {}
        for stream in ("L", "H"):
            idx, sel = packed[c][stream]
            m[f"gidx{stream}"] = _wrap_idx(idx)
            m[f"sel{stream}"] = sel.reshape(P, -1).copy()
        em = dst_core == c
        lp = dst_pos[em] - c * NPU
        ct = np.zeros((NCOMB, NPU), np.float32)
        np.add.at(ct, (cidx[em], lp), 1.0)
        m["countT"] = ct.astype(HNP)
        dg = np.zeros(NPU, np.float32)
        rp = node_at_pos[c] >= 0
        dg[rp] = deginv_all[node_at_pos[c][rp]]
        m["deginv"] = np.tile(dg[None, :], (P, 1)).astype(ml_dtypes.bfloat16)
        sp = np.zeros((NPU, GPC), np.float32)
        gl = np.where(rp)[0]
        gid = graph_ids[node_at_pos[c][gl]] - c * GPC
        cnts = np.maximum(gcnt[c * GPC:(c + 1) * GPC], 1.0)
        sp[gl, gid] = (1.0 / cnts[gid]).astype(np.float32)
        m["selpool"] = sp
        cnt9 = np.zeros((P, AC, NPU), HNP)
        pos_r = np.where(rp)[0]
        nf = nfeat[node_at_pos[c][pos_r]]
        for a in range(AC):
            cnt9[nf[:, a], a, pos_r] = 1.0
        m["cnt9"] = cnt9.reshape(P, -1).copy()
        m["atom9"] = atom9_q
        m["t512"] = T512_q
        m["wl"] = W.astype(ml_dtypes.bfloat16)             # [L, D, D]
        m["gam"] = gamma.reshape(L, 1, D).copy()
        m["bet"] = beta.reshape(L, 1, D).copy()
        m["wp"] = Wp.copy()
        m["bpr"] = bp.reshape(1, OUT).copy()
        m["ident"] = np.eye(P, dtype=HNP)
        in_maps.append(m)

    meta = dict(S=S, Nc=Nc)
    return cfg, in_maps, meta


# ----------------------------------------------------------------------------
# Device kernel builder (uniform SPMD program)
# ----------------------------------------------------------------------------

def build(cfg):
    D = cfg["D"]; L = cfg["L"]; NPU = cfg["NPU"]; NSW = cfg["NSW"]
    NWIN = cfg["NWIN"]; SPW = cfg["SPW"]; TT_L = cfg["TT_L"]; TT_H = cfg["TT_H"]
    NCOMB = cfg["NCOMB"]; AC = cfg["AC"]; GPC = cfg["GPC"]; OUT = cfg["OUT"]
    B_SPLIT = cfg["B_SPLIT"]; NREAL = cfg["NREAL"]
    KD = D // P
    NKC = NCOMB // P
    NT_L, NT_H = NSW * TT_L, NSW * TT_H
    WT_L, WT_H = SPW * TT_L, SPW * TT_H     # gather tiles per window
    NQ = int(os.environ.get("KGCN_NQ", "4"))
    PREP = os.environ.get("KGCN_PREP", "0") == "1"
    NPRE = 3                                 # windows prepped ahead at layer start

    nc = bacc.Bacc("TRN2", target_bir_lowering=False, debug=False,
                   num_devices=NCORES, num_swdge_queues=NQ)

    def allgather(ins, outs):
        if USE_FP8:
            ins = [ap.bitcast(BF16) for ap in ins]
            outs = [ap.bitcast(BF16) for ap in outs]
        nc.gpsimd.collective_compute(
            "AllGather", mybir.AluOpType.bypass,
            replica_groups=[list(range(NCORES))], ins=ins, outs=outs)

    def allgather_f32(ins, outs):
        nc.gpsimd.collective_compute(
            "AllGather", mybir.AluOpType.bypass,
            replica_groups=[list(range(NCORES))], ins=ins, outs=outs)

    t_gidxL = nc.dram_tensor("gidxL", [P, NT_L * P // 16], I16, kind="ExternalInput")
    t_gidxH = nc.dram_tensor("gidxH", [P, NT_H * P // 16], I16, kind="ExternalInput")
    t_selL = nc.dram_tensor("selL", [P, NT_L * P], HDT, kind="ExternalInput")
    t_selH = nc.dram_tensor("selH", [P, NT_H * P], HDT, kind="ExternalInput")
    t_countT = nc.dram_tensor("countT", [NCOMB, NPU], HDT, kind="ExternalInput")
    t_deginv = nc.dram_tensor("deginv", [P, NPU], BF16, kind="ExternalInput")
    t_selpool = nc.dram_tensor("selpool", [NPU, GPC], F32, kind="ExternalInput")
    t_cnt9 = nc.dram_tensor("cnt9", [P, AC * NPU], HDT, kind="ExternalInput")
    t_atom9 = nc.dram_tensor("atom9", [P, AC, D], HDT, kind="ExternalInput")
    t_t512 = nc.dram_tensor("t512", [L, NCOMB, D], HDT, kind="ExternalInput")
    t_wl = nc.dram_tensor("wl", [L, D, D], BF16, kind="ExternalInput")
    t_gam = nc.dram_tensor("gam", [L, 1, D], F32, kind="ExternalInput")
    t_bet = nc.dram_tensor("bet", [L, 1, D], F32, kind="ExternalInput")
    t_wp = nc.dram_tensor("wp", [D, OUT], F32, kind="ExternalInput")
    t_bp = nc.dram_tensor("bpr", [1, OUT], F32, kind="ExternalInput")
    t_ident = nc.dram_tensor("ident", [P, P], HDT, kind="ExternalInput")
    t_out = nc.dram_tensor("out_g", [GPC, OUT], F32, kind="ExternalOutput")
    t_hfull = nc.dram_tensor("h_full", [NCORES * NPU, D], HDT, addr_space="Shared")
    t_hnew = nc.dram_tensor("h_newc", [NPU, D], HDT)
    t_arin = [nc.dram_tensor(f"arin{l}", [1, 2 * D], F32) for l in range(L)]
    t_arout = [nc.dram_tensor(f"arout{l}", [NCORES, 2 * D], F32,
                              addr_space="Shared") for l in range(L)]

    dma_sems = [nc.alloc_semaphore(f"swdge_dma{q}") for q in range(NQ)]

    def wqueues(w):
        qa = (2 * w) % NQ
        return qa, qa + 1

    with tile.TileContext(nc) as tc:
        with (
            tc.tile_pool(name="static", bufs=1) as stp,
            tc.tile_pool(name="gath", bufs=3) as gpool,
            tc.tile_pool(name="selp", bufs=2) as selpool_p,
            tc.tile_pool(name="xt", bufs=2) as xtp,
            tc.tile_pool(name="work", bufs=3) as wk,
            tc.tile_pool(name="small", bufs=1) as smp,
            tc.tile_pool(name="winps", bufs=2, space="PSUM") as wps,
            tc.tile_pool(name="hlps", bufs=1, space="PSUM") as hps,
            tc.tile_pool(name="smps", bufs=1, space="PSUM") as sps,
            tc.tile_pool(name="abps", bufs=1, space="PSUM") as aps,
        ):
            # ---- static SBUF preloads ----
            atom9S = stp.tile([P, AC, D], HDT)
            selpS = stp.tile([P, NSW, GPC], F32)
            dgS = stp.tile([P, NPU], BF16)
            wS = stp.tile([P, L, KD, D], BF16)
            t5S = stp.tile([P, L, NKC, D], HDT)
            gamS = stp.tile([1, L, D], F32)
            betS = stp.tile([1, L, D], F32)
            wpS = stp.tile([P, KD, OUT], F32)
            bpS = stp.tile([1, OUT], F32)
            onesS = stp.tile([1, P], F32)
            onecol = stp.tile([P, 1], BF16)
            identS = stp.tile([P, P], HDT)
            hlinS = stp.tile([P, NSW, D], BF16)
            hnbS = stp.tile([P, NSW, D], HDT)
            epsS = stp.tile([1, 1], F32)
            nc.vector.memset(epsS[:], EPS)
            nc.sync.dma_start(atom9S[:], t_atom9[:])
            nc.sync.dma_start(identS[:], t_ident[:])
            nc.sync.dma_start(selpS[:], t_selpool.ap().rearrange("(s p) g -> p s g", p=P))
            nc.sync.dma_start(dgS[:], t_deginv[:])
            nc.sync.dma_start(wS[:], t_wl.ap().rearrange("l (k p) d -> p l k d", p=P))
            nc.sync.dma_start(t5S[:], t_t512.ap().rearrange("l (k p) d -> p l k d", p=P))
            nc.sync.dma_start(gamS[:], t_gam.ap().rearrange("l o d -> o l d"))
            nc.sync.dma_start(betS[:], t_bet.ap().rearrange("l o d -> o l d"))
            nc.sync.dma_start(wpS[:], t_wp.ap().rearrange("(k p) o -> p k o", p=P))
            nc.sync.dma_start(bpS[:], t_bp[:])
            nc.vector.memset(onesS[:], 1.0)
            nc.vector.memset(onecol[:], 1.0)
            ones8 = stp.tile([NCORES, 1], F32)
            nc.vector.memset(ones8[:], 1.0)

            # ============ h0: atom embedding sums via count matmuls ============
            for w in range(NWIN):
                cnt = wk.tile([P, AC, WSZ], HDT, tag="cnt", bufs=2)
                nc.sync.dma_start(
                    cnt[:], t_cnt9.ap().rearrange("p (a n) -> p a n", a=AC)
                    [:, :, w * WSZ:(w + 1) * WSZ])
                for sw in range(SPW):
                    st = w * SPW + sw
                    h0p = hps.tile([P, D], F32, tag="hl")
                    for a in range(AC):
                        nc.tensor.matmul(
                            out=h0p[:],
                            lhsT=cnt[:, a, sw * P:(sw + 1) * P],
                            rhs=atom9S[:, a, :],
                            start=(a == 0), stop=(a == AC - 1))
                    nc.scalar.activation(hnbS[:, st, :], h0p[:],
                                         mybir.ActivationFunctionType.Copy)
                    nc.sync.dma_start(t_hnew[st * P:(st + 1) * P, :],
                                      hnbS[:, st, :])

            # ================= layers =================
            def emit_prep(w, stream, gt):
                wt = WT_L if stream == "L" else WT_H
                tg = t_gidxL if stream == "L" else t_gidxH
                nidx = wt * P
                gidx = gpool.tile([P, nidx // 16], I16, tag=f"i{stream}",
                                  name=f"i{stream}t")
                nc.sync.dma_start(
                    gidx[:], tg[:, w * (nidx // 16):(w + 1) * (nidx // 16)])
                tbl = (t_hfull[0:B_SPLIT, :] if stream == "L"
                       else t_hfull[B_SPLIT:NCORES * NPU, :])
                qa, qb = wqueues(w)
                q = qa if stream == "L" else qb
                if PREP:
                    nc.gpsimd.dma_gather(
                        gt[:], tbl, gidx[:],
                        nidx, nidx, D, single_packet=False,
                        prepare_only=True, sem=dma_sems[q], queue_num=q)
                else:
                    nc.gpsimd.dma_gather(
                        gt[:], tbl, gidx[:],
                        nidx, nidx, D, single_packet=False, queue_num=q)
                return q

            def new_gt(stream):
                wt = WT_L if stream == "L" else WT_H
                return gpool.tile([P, wt, D], HDT, tag=f"g{stream}",
                                  name=f"g{stream}t")

            def new_sel(w, stream):
                wt = WT_L if stream == "L" else WT_H
                tsel = t_selL if stream == "L" else t_selH
                sel = selpool_p.tile([P, wt, P], HDT, tag=f"s{stream}",
                                     name=f"s{stream}t")
                nc.sync.dma_start(
                    sel[:], tsel[:, w * (wt * P):(w + 1) * (wt * P)])
                return sel

            def new_ctk(w):
                ctk = wk.tile([P, NKC, WSZ], HDT, tag="ct", bufs=2)
                nc.sync.dma_start(
                    ctk[:], t_countT.ap().rearrange(
                        "(k p) n -> p k n", p=P)[:, :, w * WSZ:(w + 1) * WSZ])
                return ctk

            for l in range(L):
                # prologue: prep the first windows' gathers so descriptor gen
                # runs during the previous layer's AR/apply and this AG.
                # Only the triggers carry the h_full dependency.
                pre_gt = {}
                pre_sel = {}
                pre_ctk = {}
                if PREP:
                    for w in range(NPRE - 1):
                        for stream in ("L", "H"):
                            gt = new_gt(stream)
                            emit_prep(w, stream, gt)
                            pre_gt[(w, stream)] = gt
                allgather([t_hnew[:]], [t_hfull[:]])
                if PREP:
                    for stream in ("L", "H"):
                        gt = new_gt(stream)
                        emit_prep(NPRE - 1, stream, gt)
                        pre_gt[(NPRE - 1, stream)] = gt
                    for q in range(NQ):
                        nc.gpsimd.trigger_dma(count=None, queue_num=q)
                for w in range(NPRE - 1):
                    pre_ctk[w] = new_ctk(w)
                    for stream in ("L", "H"):
                        pre_sel[(w, stream)] = new_sel(w, stream)
                stats0 = sps.tile([1, D], F32, tag="stats0")
                stats1 = sps.tile([1, D], F32, tag="stats1")
                if l == L - 1:
                    poolps = [sps.tile([P, GPC], F32, tag=f"pool{h}",
                                       name=f"pool{h}") for h in range(KD)]
                for w in range(NWIN):
                    winp = [wps.tile([P, WSZ], F32, tag="win", name=f"win{h}")
                            for h in range(KD)]
                    ctk = pre_ctk.pop(w) if w in pre_ctk else new_ctk(w)
                    gts = {}
                    sels = {}
                    for stream in ("L", "H"):
                        if (w, stream) in pre_gt:
                            gts[stream] = pre_gt.pop((w, stream))
                        else:
                            gt = new_gt(stream)
                            q = emit_prep(w, stream, gt)
                            if PREP:
                                nc.gpsimd.trigger_dma(count=None, queue_num=q)
                            gts[stream] = gt
                        if (w, stream) in pre_sel:
                            sels[stream] = pre_sel.pop((w, stream))
                        else:
                            sels[stream] = new_sel(w, stream)
                    # bond term: window-wide, starts the PSUM accumulation
                    for kk in range(NKC):
                        for h in range(KD):
                            nc.tensor.matmul(
                                out=winp[h][:],
                                lhsT=t5S[:, l, kk, h * P:(h + 1) * P],
                                rhs=ctk[:, kk, :],
                                start=(kk == 0), stop=False)
                    for sw in range(SPW):
                        st = w * SPW + sw
                        # self term via identity (hnbS holds this layer's input)
                        for h in range(KD):
                            nc.tensor.matmul(
                                out=winp[h][:, sw * P:(sw + 1) * P],
                                lhsT=hnbS[:, st, h * P:(h + 1) * P],
                                rhs=identS[:],
                                start=False, stop=False)
                        for stream, tt in (("L", TT_L), ("H", TT_H)):
                            gt = gts[stream]
                            sel = sels[stream]
                            last_stream = stream == "H"
                            for t in range(tt):
                                ti = sw * tt + t
                                for h in range(KD):
                                    nc.tensor.matmul(
                                        out=winp[h][:, sw * P:(sw + 1) * P],
                                        lhsT=gt[:, ti, h * P:(h + 1) * P],
                                        rhs=sel[:, ti, :],
                                        start=False,
                                        stop=(last_stream and t == tt - 1))
                    # x^T = deginv * window  (bf16)
                    xt = [xtp.tile([P, WSZ], BF16, tag="xt", name=f"xt{h}")
                          for h in range(KD)]
                    for h in range(KD):
                        nc.vector.tensor_tensor(
                            out=xt[h][:], in0=winp[h][:],
                            in1=dgS[:, w * WSZ:(w + 1) * WSZ],
                            op=mybir.AluOpType.mult)
                    # update matmul + stats per subtile
                    for sw in range(SPW):
                        st = w * SPW + sw
                        hlp = hps.tile([P, D], F32, tag="hl")
                        for h in range(KD):
                            nc.tensor.matmul(
                                out=hlp[:],
                                lhsT=xt[h][:, sw * P:(sw + 1) * P],
                                rhs=wS[:, l, h, :],
                                start=(h == 0), stop=(h == KD - 1))
                        nc.scalar.activation(hlinS[:, st, :], hlp[:],
                                             mybir.ActivationFunctionType.Copy)
                        sq = wk.tile([P, D], BF16, tag="sq")
                        nc.vector.tensor_tensor(out=sq[:], in0=hlinS[:, st, :],
                                                in1=hlinS[:, st, :],
                                                op=mybir.AluOpType.mult)
                        nc.tensor.matmul(out=stats0[:],
                                         lhsT=onecol[:],
                                         rhs=hlinS[:, st, :],
                                         start=(st == 0), stop=(st == NSW - 1))
                        nc.tensor.matmul(out=stats1[:],
                                         lhsT=onecol[:], rhs=sq[:],
                                         start=(st == 0), stop=(st == NSW - 1))
                # --- BN stats: AllGather per-core stats, reduce locally ---
                stsb = smp.tile([1, 2 * D], F32, tag="stsb")
                nc.scalar.activation(stsb[:, 0:D], stats0[:],
                                     mybir.ActivationFunctionType.Copy)
                nc.scalar.activation(stsb[:, D:2 * D], stats1[:],
                                     mybir.ActivationFunctionType.Copy)
                nc.sync.dma_start(t_arin[l][:], stsb[:])
                allgather_f32([t_arin[l][:]], [t_arout[l][:]])
                stg = smp.tile([NCORES, 2 * D], F32, tag="stg")
                nc.sync.dma_start(stg[:], t_arout[l][:])
                nc.tensor.matmul(out=stats0[:], lhsT=ones8[:],
                                 rhs=stg[:, 0:D], start=True, stop=True)
                nc.tensor.matmul(out=stats1[:], lhsT=ones8[:],
                                 rhs=stg[:, D:2 * D], start=True, stop=True)
                mean = smp.tile([1, D], F32, tag="mean")
                nc.vector.tensor_scalar_mul(mean[:], stats0[:], 1.0 / NREAL)
                msq = smp.tile([1, D], F32, tag="msq")
                nc.vector.tensor_scalar_mul(msq[:], stats1[:], 1.0 / NREAL)
                var = smp.tile([1, D], F32, tag="var")
                nc.vector.tensor_tensor(out=var[:], in0=mean[:], in1=mean[:],
                                        op=mybir.AluOpType.mult)
                nc.vector.tensor_tensor(out=var[:], in0=msq[:], in1=var[:],
                                        op=mybir.AluOpType.subtract)
                sd = smp.tile([1, D], F32, tag="sd")
                nc.scalar.activation(sd[:], var[:],
                                     mybir.ActivationFunctionType.Sqrt,
                                     bias=epsS[:])
                rsq = smp.tile([1, D], F32, tag="rsq")
                nc.vector.reciprocal(rsq[:], sd[:])
                scl = smp.tile([1, D], F32, tag="scl")
                nc.vector.tensor_tensor(out=scl[:], in0=rsq[:],
                                        in1=gamS[:, l, :],
                                        op=mybir.AluOpType.mult)
                sft = smp.tile([1, D], F32, tag="sft")
                nc.vector.tensor_tensor(out=sft[:], in0=mean[:], in1=scl[:],
                                        op=mybir.AluOpType.mult)
                nc.vector.tensor_tensor(out=sft[:], in0=betS[:, l, :],
                                        in1=sft[:],
                                        op=mybir.AluOpType.subtract)
                ab = aps.tile([P, 2 * D], F32, tag="ab")
                nc.tensor.matmul(out=ab[:, 0:D], lhsT=onesS[:], rhs=scl[:],
                                 start=True, stop=True)
                nc.tensor.matmul(out=ab[:, D:2 * D], lhsT=onesS[:], rhs=sft[:],
                                 start=True, stop=True)
                # --- apply (window-batched) + (layer L-1) pooling ---
                a0 = ab[:, 0:D]
                a0b = bass.AP(a0.tensor, a0.offset,
                              [a0.ap[0], [0, SPW], a0.ap[1]])
                a1 = ab[:, D:2 * D]
                a1b = bass.AP(a1.tensor, a1.offset,
                              [a1.ap[0], [0, SPW], a1.ap[1]])
                for w in range(NWIN):
                    hnf = wk.tile([P, SPW, D], F32, tag="hnf", bufs=2)
                    nc.vector.tensor_tensor(
                        out=hnf[:], in0=hlinS[:, w * SPW:(w + 1) * SPW, :],
                        in1=a0b, op=mybir.AluOpType.mult)
                    nc.vector.tensor_tensor(out=hnf[:], in0=hnf[:],
                                            in1=a1b, op=mybir.AluOpType.add)
                    if l < L - 1:
                        nc.scalar.activation(
                            hnbS[:, w * SPW:(w + 1) * SPW, :], hnf[:],
                            mybir.ActivationFunctionType.Relu)
                        nc.sync.dma_start(
                            t_hnew.ap()[w * WSZ:(w + 1) * WSZ, :].rearrange(
                                "(s p) d -> p s d", p=P),
                            hnbS[:, w * SPW:(w + 1) * SPW, :])
                    else:
                        hnr = wk.tile([P, SPW, D], F32, tag="hnr", bufs=2)
                        nc.vector.tensor_scalar_max(hnr[:], hnf[:], 0.0)
                        for sw in range(SPW):
                            st = w * SPW + sw
                            for h in range(KD):
                                nc.tensor.matmul(
                                    out=poolps[h][:],
                                    lhsT=hnr[:, sw, h * P:(h + 1) * P],
                                    rhs=selpS[:, st, :],
                                    start=(st == 0), stop=(st == NSW - 1))

            # ================= readout =================
            gts = smp.tile([P, KD * GPC], F32, tag="gts")
            for h in range(KD):
                nc.scalar.activation(gts[:, h * GPC:(h + 1) * GPC],
                                     poolps[h][:],
                                     mybir.ActivationFunctionType.Copy)
            ones16 = smp.tile([1, GPC], F32, tag="o16")
            nc.vector.memset(ones16[:], 1.0)
            outp = sps.tile([GPC, OUT], F32, tag="stats0")
            for h in range(KD):
                nc.tensor.matmul(out=outp[:],
                                 lhsT=gts[:, h * GPC:(h + 1) * GPC],
                                 rhs=wpS[:, h, :], start=(h == 0), stop=False)
            nc.tensor.matmul(out=outp[:], lhsT=ones16[:], rhs=bpS[:],
                             start=False, stop=True)
            outs = smp.tile([GPC, OUT], F32, tag="outs")
            nc.scalar.activation(outs[:], outp[:],
                                 mybir.ActivationFunctionType.Copy)
            nc.sync.dma_start(t_out[:], outs[:])

    nc.compile()
    return nc


LAST = {}


def kernel(**inputs):
    cfg, in_maps, _ = preprocess(inputs)
    nc = build(cfg)
    trace = os.environ.get("KGCN_TRACE") == "1"
    res = run_bass_kernel_spmd(nc, in_maps, list(range(NCORES)), trace=trace)
    LAST["exec_time_ns"] = res.exec_time_ns
    LAST["profile_json"] = res.profile_json
    out = np.concatenate([res.results[c]["out_g"] for c in range(NCORES)], 0)
    return out.astype(np.float32)


if __name__ == "__main__":
    pass


# revision 44
# speedup vs baseline: 1.4080x; 1.0352x over previous
"""GCN message-passing kernel for 8 Trainium2 NeuronCores (Bass/Tile).

v2 redesign vs v1 (4.66ms -> target <2.5ms):
- fp8 h table + Shared collective outputs by default (v1 had them off).
- Gathers merged per (window, stream): 26 DMAGatherAnt/layer instead of 104
  (amortizes ~1us fixed Q7 descriptor-gen cost per instruction).
- Selection matrices precomputed on host and DMA'd as inputs (removes the
  DVE IS_EQ chain, ~700us, plus GpSimd SBUF-port contention).
- Self-loop h term no longer gathered: post-activation h kept in SBUF
  (hnbS) and added to the PSUM window via identity matmuls (-5% descriptors).
- Bond-encoder matmuls window-wide (512-free) instead of per-subwindow.
- Dead-row zeroing dropped (pad edges have all-zero sel columns; pad nodes
  have deginv=0 and zero selpool rows, so garbage never propagates).
- AllReduce output Shared.
"""
import sys

sys.path.insert(0, "/opt/trn_rl_repo")

import os

import numpy as np
import ml_dtypes

import concourse.bass as bass
import concourse.bacc as bacc
import concourse.mybir as mybir
import concourse.tile as tile
from concourse.bass_utils import run_bass_kernel_spmd

P = 128
WSZ = 512          # psum node window
EPS = 1e-5
NCORES = 8
BF16 = mybir.dt.bfloat16
FP8 = mybir.dt.float8e4
F32 = mybir.dt.float32
I16 = mybir.dt.int16
S0 = 64.0          # layer-0 table scale (absorbed by BN)

USE_FP8 = os.environ.get("KGCN_FP8", "1") == "1"
HDT = FP8 if USE_FP8 else BF16
HNP = ml_dtypes.float8_e4m3fn if USE_FP8 else ml_dtypes.bfloat16


# ----------------------------------------------------------------------------
# Host preprocessing
# ----------------------------------------------------------------------------

def _wrap_idx(flat):
    n = flat.shape[0]
    assert n % 16 == 0
    w = flat.reshape(n // 16, 16).T.astype(np.int16)  # [16, n/16]
    return np.tile(w, (8, 1))


def preprocess(inputs, n_graphs=128):
    nfeat = np.asarray(inputs["nfeat"], np.int64)
    efeat = np.asarray(inputs["efeat"], np.int64)
    src = np.asarray(inputs["src"], np.int64)
    dst = np.asarray(inputs["dst"], np.int64)
    graph_ids = np.asarray(inputs["graph_ids"], np.int64)
    atom_emb = np.asarray(inputs["atom_emb"], np.float32)
    edge_emb = np.asarray(inputs["edge_emb"], np.float32)
    W = np.asarray(inputs["W"], np.float32)
    gamma = np.asarray(inputs["gamma"], np.float32)
    beta = np.asarray(inputs["beta"], np.float32)
    Wp = np.asarray(inputs["Wp"], np.float32)
    bp = np.asarray(inputs["bp"], np.float32)

    N = graph_ids.shape[0]
    E = src.shape[0]
    G = n_graphs
    GPC = G // NCORES
    AC, AV, D = atom_emb.shape
    L, BC, BV, _ = edge_emb.shape
    NCOMB = BV ** BC
    OUT = Wp.shape[1]
    HALF = NCORES // 2

    gcnt = np.bincount(graph_ids, minlength=G)
    gofs = np.concatenate([[0], np.cumsum(gcnt)])
    S = gofs[::GPC].astype(np.int64)
    assert S[-1] == N
    Nc = np.diff(S)

    NSW = int(np.ceil((Nc.max() + 1) / P))
    NPU = NSW * P
    NWIN = NPU // WSZ
    if NWIN * WSZ < NPU:
        NWIN += 1
        NPU = NWIN * WSZ
        NSW = NPU // P
    SPW = WSZ // P
    B_SPLIT = HALF * NPU
    assert B_SPLIT < 32768 and (NCORES - HALF) * NPU < 32768

    degs = np.bincount(dst, minlength=N).astype(np.float64) + 1.0
    deginv_all = (1.0 / degs).astype(np.float32)

    node_core = np.searchsorted(S[1:], np.arange(N), side="right").astype(np.int64)
    src_core = node_core[src]
    e_isL = src_core < HALF
    dLn = np.bincount(dst[e_isL], minlength=N)
    dHn = np.bincount(dst[~e_isL], minlength=N)

    # --- per-core node permutation: balance (dL, dH) across NSW bins ---
    pos_of_node = np.full(N, -1, np.int64)
    node_at_pos = [np.full(NPU, -1, np.int64) for _ in range(NCORES)]
    for c in range(NCORES):
        nodes = np.arange(S[c], S[c + 1])
        wl = dLn[nodes].astype(np.int64)
        wh = dHn[nodes].astype(np.int64)
        order = np.argsort(-(wl + wh), kind="stable")
        binL = np.zeros(NSW, np.int64)
        binH = np.zeros(NSW, np.int64)
        binN = np.zeros(NSW, np.int64)
        for i in order:
            nl, nh = wl[i], wh[i]
            cand = np.maximum(binL + nl, binH + nh) + 1e-3 * (binL + binH)
            cand[binN >= P] = 1 << 60
            b = int(np.argmin(cand))
            slot = binN[b]
            binN[b] += 1
            binL[b] += nl
            binH[b] += nh
            n = nodes[i]
            pos_of_node[n] = c * NPU + b * P + slot
            node_at_pos[c][b * P + slot] = n

    # --- edge streams per (core, stream): edges only, sorted by dst ---
    src_pg = pos_of_node[src]
    dst_pos = pos_of_node[dst]
    dst_core = node_core[dst]

    core_streams = []   # [core][stream] -> (srcpos_sorted, dstlocal_sorted)
    for c in range(NCORES):
        em = dst_core == c
        es, ed, eL = src_pg[em], dst_pos[em] - c * NPU, e_isL[em]
        per = {}
        for stream, m in (("L", eL), ("H", ~eL)):
            ssrc = es[m]
            sdst = ed[m]
            o = np.argsort(sdst, kind="stable")
            ssrc, sdst = ssrc[o], sdst[o]
            if stream == "H":
                ssrc = ssrc - B_SPLIT
            per[stream] = (ssrc, sdst)
        core_streams.append(per)

    # tiles per (sw, stream) = global max of ceil(edges_sw / P)
    TT = {}
    for stream in ("L", "H"):
        mx = 1
        for c in range(NCORES):
            _, sdst = core_streams[c][stream]
            cnt = np.bincount(sdst // P, minlength=NSW)
            mx = max(mx, int(np.ceil(cnt.max() / P)))
        TT[stream] = mx

    def pack_core(c):
        out = {}
        for stream in ("L", "H"):
            ssrc, sdst = core_streams[c][stream]
            tt = TT[stream]
            idx = np.zeros((NSW, tt * P), np.int16)   # pad -> row 0 (sel=0)
            sel = np.zeros((P, NSW * tt, P), np.float32)
            sw_of = sdst // P
            starts = np.concatenate([[0], np.cumsum(np.bincount(sw_of, minlength=NSW))])
            for sw in range(NSW):
                r0, r1 = starts[sw], starts[sw + 1]
                k = r1 - r0
                assert k <= tt * P
                idx[sw, :k] = ssrc[r0:r1]
                rows = np.arange(k)
                sel[rows % P, sw * tt + rows // P, sdst[r0:r1] - sw * P] = 1.0
            out[stream] = (idx.reshape(-1), sel.astype(HNP))
        return out

    packed = [pack_core(c) for c in range(NCORES)]

    # --- tables ---
    # atom9[p, a, :] = atom_emb[a, p, :] * S0  (vocab entry p of column a)
    atom9_q = (np.transpose(atom_emb, (1, 0, 2)) * S0).astype(HNP)

    k = np.arange(NCOMB)
    d0, d1, d2 = k // (BV * BV), (k // BV) % BV, k % BV
    T512 = edge_emb[:, 0, d0] + edge_emb[:, 1, d1] + edge_emb[:, 2, d2]
    T512[0] *= S0
    T512_q = T512.astype(HNP)

    cidx = (efeat[:, 0] * BV + efeat[:, 1]) * BV + efeat[:, 2]

    cfg = dict(N=N, E=E, G=G, GPC=GPC, D=D, L=L, OUT=OUT, NPU=NPU, NSW=NSW,
               NWIN=NWIN, SPW=SPW, TT_L=TT["L"], TT_H=TT["H"],
               B_SPLIT=B_SPLIT, NCOMB=NCOMB, AC=AC, NREAL=N)

    in_maps = []
    for c in range(NCORES):
        m = {}
        for stream in ("L", "H"):
            idx, sel = packed[c][stream]
            m[f"gidx{stream}"] = _wrap_idx(idx)
            m[f"sel{stream}"] = sel.reshape(P, -1).copy()
        em = dst_core == c
        lp = dst_pos[em] - c * NPU
        ct = np.zeros((NCOMB, NPU), np.float32)
        np.add.at(ct, (cidx[em], lp), 1.0)
        m["countT"] = ct.astype(HNP)
        dg = np.zeros(NPU, np.float32)
        rp = node_at_pos[c] >= 0
        dg[rp] = deginv_all[node_at_pos[c][rp]]
        m["deginv"] = np.tile(dg[None, :], (P, 1)).astype(ml_dtypes.bfloat16)
        sp = np.zeros((NPU, GPC), np.float32)
        gl = np.where(rp)[0]
        gid = graph_ids[node_at_pos[c][gl]] - c * GPC
        cnts = np.maximum(gcnt[c * GPC:(c + 1) * GPC], 1.0)
        sp[gl, gid] = (1.0 / cnts[gid]).astype(np.float32)
        m["selpool"] = sp
        cnt9 = np.zeros((P, AC, NPU), HNP)
        pos_r = np.where(rp)[0]
        nf = nfeat[node_at_pos[c][pos_r]]
        for a in range(AC):
            cnt9[nf[:, a], a, pos_r] = 1.0
        m["cnt9"] = cnt9.reshape(P, -1).copy()
        m["atom9"] = atom9_q
        m["t512"] = T512_q
        m["wl"] = W.astype(ml_dtypes.bfloat16)             # [L, D, D]
        m["gam"] = gamma.reshape(L, 1, D).copy()
        m["bet"] = beta.reshape(L, 1, D).copy()
        m["wp"] = Wp.copy()
        m["bpr"] = bp.reshape(1, OUT).copy()
        m["ident"] = np.eye(P, dtype=HNP)
        m["dzero"] = np.zeros((P, P // 16), np.int16)
        in_maps.append(m)

    meta = dict(S=S, Nc=Nc)
    return cfg, in_maps, meta


# ----------------------------------------------------------------------------
# Device kernel builder (uniform SPMD program)
# ----------------------------------------------------------------------------

def build(cfg):
    D = cfg["D"]; L = cfg["L"]; NPU = cfg["NPU"]; NSW = cfg["NSW"]
    NWIN = cfg["NWIN"]; SPW = cfg["SPW"]; TT_L = cfg["TT_L"]; TT_H = cfg["TT_H"]
    NCOMB = cfg["NCOMB"]; AC = cfg["AC"]; GPC = cfg["GPC"]; OUT = cfg["OUT"]
    B_SPLIT = cfg["B_SPLIT"]; NREAL = cfg["NREAL"]
    KD = D // P
    NKC = NCOMB // P
    NT_L, NT_H = NSW * TT_L, NSW * TT_H
    WT_L, WT_H = SPW * TT_L, SPW * TT_H     # gather tiles per window
    NQ = int(os.environ.get("KGCN_NQ", "4"))
    PREP = os.environ.get("KGCN_PREP", "0") == "1"
    NPRE = 3                                 # windows prepped ahead at layer start

    nc = bacc.Bacc("TRN2", target_bir_lowering=False, debug=False,
                   num_devices=NCORES, num_swdge_queues=NQ)

    def allgather(ins, outs):
        if USE_FP8:
            ins = [ap.bitcast(BF16) for ap in ins]
            outs = [ap.bitcast(BF16) for ap in outs]
        nc.gpsimd.collective_compute(
            "AllGather", mybir.AluOpType.bypass,
            replica_groups=[list(range(NCORES))], ins=ins, outs=outs)

    def allreduce(ins, outs):
        nc.gpsimd.collective_compute(
            "AllReduce", mybir.AluOpType.add,
            replica_groups=[list(range(NCORES))], ins=ins, outs=outs)

    t_gidxL = nc.dram_tensor("gidxL", [P, NT_L * P // 16], I16, kind="ExternalInput")
    t_gidxH = nc.dram_tensor("gidxH", [P, NT_H * P // 16], I16, kind="ExternalInput")
    t_selL = nc.dram_tensor("selL", [P, NT_L * P], HDT, kind="ExternalInput")
    t_selH = nc.dram_tensor("selH", [P, NT_H * P], HDT, kind="ExternalInput")
    t_countT = nc.dram_tensor("countT", [NCOMB, NPU], HDT, kind="ExternalInput")
    t_deginv = nc.dram_tensor("deginv", [P, NPU], BF16, kind="ExternalInput")
    t_selpool = nc.dram_tensor("selpool", [NPU, GPC], F32, kind="ExternalInput")
    t_cnt9 = nc.dram_tensor("cnt9", [P, AC * NPU], HDT, kind="ExternalInput")
    t_atom9 = nc.dram_tensor("atom9", [P, AC, D], HDT, kind="ExternalInput")
    t_t512 = nc.dram_tensor("t512", [L, NCOMB, D], HDT, kind="ExternalInput")
    t_wl = nc.dram_tensor("wl", [L, D, D], BF16, kind="ExternalInput")
    t_gam = nc.dram_tensor("gam", [L, 1, D], F32, kind="ExternalInput")
    t_bet = nc.dram_tensor("bet", [L, 1, D], F32, kind="ExternalInput")
    t_wp = nc.dram_tensor("wp", [D, OUT], F32, kind="ExternalInput")
    t_bp = nc.dram_tensor("bpr", [1, OUT], F32, kind="ExternalInput")
    t_ident = nc.dram_tensor("ident", [P, P], HDT, kind="ExternalInput")
    t_dzero = nc.dram_tensor("dzero", [P, P // 16], I16, kind="ExternalInput")
    t_out = nc.dram_tensor("out_g", [GPC, OUT], F32, kind="ExternalOutput")
    t_hfull = nc.dram_tensor("h_full", [NCORES * NPU, D], HDT, addr_space="Shared")
    t_hnew = nc.dram_tensor("h_newc", [NPU, D], HDT)
    t_arin = [nc.dram_tensor(f"arin{l}", [1, 2 * D], F32) for l in range(L)]
    t_arout = [nc.dram_tensor(f"arout{l}", [1, 2 * D], F32,
                              addr_space="Shared") for l in range(L)]

    dma_sems = [nc.alloc_semaphore(f"swdge_dma{q}") for q in range(NQ)]

    def wqueues(w):
        qa = (2 * w) % NQ
        return qa, qa + 1

    with tile.TileContext(nc) as tc:
        with (
            tc.tile_pool(name="static", bufs=1) as stp,
            tc.tile_pool(name="gath", bufs=3) as gpool,
            tc.tile_pool(name="selp", bufs=2) as selpool_p,
            tc.tile_pool(name="xt", bufs=2) as xtp,
            tc.tile_pool(name="work", bufs=3) as wk,
            tc.tile_pool(name="small", bufs=1) as smp,
            tc.tile_pool(name="winps", bufs=2, space="PSUM") as wps,
            tc.tile_pool(name="hlps", bufs=1, space="PSUM") as hps,
            tc.tile_pool(name="smps", bufs=1, space="PSUM") as sps,
            tc.tile_pool(name="abps", bufs=1, space="PSUM") as aps,
        ):
            # ---- static SBUF preloads ----
            atom9S = stp.tile([P, AC, D], HDT)
            selpS = stp.tile([P, NSW, GPC], F32)
            dgS = stp.tile([P, NPU], BF16)
            wS = stp.tile([P, L, KD, D], BF16)
            t5S = stp.tile([P, L, NKC, D], HDT)
            gamS = stp.tile([1, L, D], F32)
            betS = stp.tile([1, L, D], F32)
            wpS = stp.tile([P, KD, OUT], F32)
            bpS = stp.tile([1, OUT], F32)
            onesS = stp.tile([1, P], F32)
            onecol = stp.tile([P, 1], BF16)
            identS = stp.tile([P, P], HDT)
            hlinS = stp.tile([P, NSW, D], BF16)
            hnbS = stp.tile([P, NSW, D], HDT)
            epsS = stp.tile([1, 1], F32)
            nc.vector.memset(epsS[:], EPS)
            nc.sync.dma_start(atom9S[:], t_atom9[:])
            nc.sync.dma_start(identS[:], t_ident[:])
            nc.sync.dma_start(selpS[:], t_selpool.ap().rearrange("(s p) g -> p s g", p=P))
            nc.sync.dma_start(dgS[:], t_deginv[:])
            nc.sync.dma_start(wS[:], t_wl.ap().rearrange("l (k p) d -> p l k d", p=P))
            nc.sync.dma_start(t5S[:], t_t512.ap().rearrange("l (k p) d -> p l k d", p=P))
            nc.sync.dma_start(gamS[:], t_gam.ap().rearrange("l o d -> o l d"))
            nc.sync.dma_start(betS[:], t_bet.ap().rearrange("l o d -> o l d"))
            nc.sync.dma_start(wpS[:], t_wp.ap().rearrange("(k p) o -> p k o", p=P))
            nc.sync.dma_start(bpS[:], t_bp[:])
            nc.vector.memset(onesS[:], 1.0)
            nc.vector.memset(onecol[:], 1.0)
            dzeroS = stp.tile([P, P // 16], I16)
            nc.sync.dma_start(dzeroS[:], t_dzero[:])

            # ============ h0: atom embedding sums via count matmuls ============
            for w in range(NWIN):
                cnt = wk.tile([P, AC, WSZ], HDT, tag="cnt", bufs=2)
                nc.sync.dma_start(
                    cnt[:], t_cnt9.ap().rearrange("p (a n) -> p a n", a=AC)
                    [:, :, w * WSZ:(w + 1) * WSZ])
                for sw in range(SPW):
                    st = w * SPW + sw
                    h0p = hps.tile([P, D], F32, tag="hl")
                    for a in range(AC):
                        nc.tensor.matmul(
                            out=h0p[:],
                            lhsT=cnt[:, a, sw * P:(sw + 1) * P],
                            rhs=atom9S[:, a, :],
                            start=(a == 0), stop=(a == AC - 1))
                    nc.scalar.activation(hnbS[:, st, :], h0p[:],
                                         mybir.ActivationFunctionType.Copy)
                    nc.sync.dma_start(t_hnew[st * P:(st + 1) * P, :],
                                      hnbS[:, st, :])

            # ================= layers =================
            def emit_prep(w, stream, gt, prep=True):
                wt = WT_L if stream == "L" else WT_H
                tg = t_gidxL if stream == "L" else t_gidxH
                nidx = wt * P
                gidx = gpool.tile([P, nidx // 16], I16, tag=f"i{stream}",
                                  name=f"i{stream}t")
                nc.sync.dma_start(
                    gidx[:], tg[:, w * (nidx // 16):(w + 1) * (nidx // 16)])
                tbl = (t_hfull[0:B_SPLIT, :] if stream == "L"
                       else t_hfull[B_SPLIT:NCORES * NPU, :])
                qa, qb = wqueues(w)
                q = qa if stream == "L" else qb
                if prep:
                    nc.gpsimd.dma_gather(
                        gt[:], tbl, gidx[:],
                        nidx, nidx, D, single_packet=False,
                        prepare_only=True, sem=dma_sems[q], queue_num=q)
                else:
                    nc.gpsimd.dma_gather(
                        gt[:], tbl, gidx[:],
                        nidx, nidx, D, single_packet=False, queue_num=q)
                return q

            def new_gt(stream):
                wt = WT_L if stream == "L" else WT_H
                return gpool.tile([P, wt, D], HDT, tag=f"g{stream}",
                                  name=f"g{stream}t")

            def new_sel(w, stream):
                wt = WT_L if stream == "L" else WT_H
                tsel = t_selL if stream == "L" else t_selH
                sel = selpool_p.tile([P, wt, P], HDT, tag=f"s{stream}",
                                     name=f"s{stream}t")
                nc.sync.dma_start(
                    sel[:], tsel[:, w * (wt * P):(w + 1) * (wt * P)])
                return sel

            def new_ctk(w):
                ctk = wk.tile([P, NKC, WSZ], HDT, tag="ct", bufs=2)
                nc.sync.dma_start(
                    ctk[:], t_countT.ap().rearrange(
                        "(k p) n -> p k n", p=P)[:, :, w * WSZ:(w + 1) * WSZ])
                return ctk

            def shadow_preps(pre_gt):
                # Shadow preps for the NEXT layer's first windows: traced
                # before the BN stats/AR/apply tail, so the Pool engine
                # generates their descriptors during that tail.  The ring
                # entries stay untriggered until fire_shadow().
                if not PREP:
                    return
                for w in range(NPRE):
                    for stream in ("L", "H"):
                        gt = new_gt(stream)
                        emit_prep(w, stream, gt)
                        pre_gt[(w, stream)] = gt

            def fire_shadow():
                # Dummy preps traced AFTER the AllGather: their deferred
                # h_full read binds the triggers to the fresh table, gating
                # the shadow preps' DMAs correctly.
                for q in range(NQ):
                    dgt = gpool.tile([P, 1, D], HDT, tag="gd", name="gdt",
                                     bufs=2)
                    nc.gpsimd.dma_gather(
                        dgt[:], t_hfull[0:B_SPLIT, :], dzeroS[:], P, P, D,
                        single_packet=False, prepare_only=True,
                        sem=dma_sems[q], queue_num=q)
                for q in range(NQ):
                    nc.gpsimd.trigger_dma(count=None, queue_num=q)

            pre_gt = {}
            shadow_preps(pre_gt)

            for l in range(L):
                allgather([t_hnew[:]], [t_hfull[:]])
                if pre_gt:
                    fire_shadow()
                pre_sel = {}
                pre_ctk = {}
                for w in range(NPRE - 1):
                    pre_ctk[w] = new_ctk(w)
                    for stream in ("L", "H"):
                        pre_sel[(w, stream)] = new_sel(w, stream)
                stats0 = sps.tile([1, D], F32, tag="stats0")
                stats1 = sps.tile([1, D], F32, tag="stats1")
                if l == L - 1:
                    poolps = [sps.tile([P, GPC], F32, tag=f"pool{h}",
                                       name=f"pool{h}") for h in range(KD)]
                for w in range(NWIN):
                    winp = [wps.tile([P, WSZ], F32, tag="win", name=f"win{h}")
                            for h in range(KD)]
                    ctk = pre_ctk.pop(w) if w in pre_ctk else new_ctk(w)
                    gts = {}
                    sels = {}
                    for stream in ("L", "H"):
                        if (w, stream) in pre_gt:
                            gts[stream] = pre_gt.pop((w, stream))
                        else:
                            gt = new_gt(stream)
                            emit_prep(w, stream, gt, prep=False)
                            gts[stream] = gt
                        if (w, stream) in pre_sel:
                            sels[stream] = pre_sel.pop((w, stream))
                        else:
                            sels[stream] = new_sel(w, stream)
                    # bond term: window-wide, starts the PSUM accumulation
                    for kk in range(NKC):
                        for h in range(KD):
                            nc.tensor.matmul(
                                out=winp[h][:],
                                lhsT=t5S[:, l, kk, h * P:(h + 1) * P],
                                rhs=ctk[:, kk, :],
                                start=(kk == 0), stop=False)
                    for sw in range(SPW):
                        st = w * SPW + sw
                        # self term via identity (hnbS holds this layer's input)
                        for h in range(KD):
                            nc.tensor.matmul(
                                out=winp[h][:, sw * P:(sw + 1) * P],
                                lhsT=hnbS[:, st, h * P:(h + 1) * P],
                                rhs=identS[:],
                                start=False, stop=False)
                        for stream, tt in (("L", TT_L), ("H", TT_H)):
                            gt = gts[stream]
                            sel = sels[stream]
                            last_stream = stream == "H"
                            for t in range(tt):
                                ti = sw * tt + t
                                for h in range(KD):
                                    nc.tensor.matmul(
                                        out=winp[h][:, sw * P:(sw + 1) * P],
                                        lhsT=gt[:, ti, h * P:(h + 1) * P],
                                        rhs=sel[:, ti, :],
                                        start=False,
                                        stop=(last_stream and t == tt - 1))
                    # x^T = deginv * window  (bf16)
                    xt = [xtp.tile([P, WSZ], BF16, tag="xt", name=f"xt{h}")
                          for h in range(KD)]
                    for h in range(KD):
                        nc.vector.tensor_tensor(
                            out=xt[h][:], in0=winp[h][:],
                            in1=dgS[:, w * WSZ:(w + 1) * WSZ],
                            op=mybir.AluOpType.mult)
                    # update matmul + stats per subtile
                    for sw in range(SPW):
                        st = w * SPW + sw
                        hlp = hps.tile([P, D], F32, tag="hl")
                        for h in range(KD):
                            nc.tensor.matmul(
                                out=hlp[:],
                                lhsT=xt[h][:, sw * P:(sw + 1) * P],
                                rhs=wS[:, l, h, :],
                                start=(h == 0), stop=(h == KD - 1))
                        nc.scalar.activation(hlinS[:, st, :], hlp[:],
                                             mybir.ActivationFunctionType.Copy)
                        sq = wk.tile([P, D], BF16, tag="sq")
                        nc.vector.tensor_tensor(out=sq[:], in0=hlinS[:, st, :],
                                                in1=hlinS[:, st, :],
                                                op=mybir.AluOpType.mult)
                        nc.tensor.matmul(out=stats0[:],
                                         lhsT=onecol[:],
                                         rhs=hlinS[:, st, :],
                                         start=(st == 0), stop=(st == NSW - 1))
                        nc.tensor.matmul(out=stats1[:],
                                         lhsT=onecol[:], rhs=sq[:],
                                         start=(st == 0), stop=(st == NSW - 1))
                if l < L - 1:
                    shadow_preps(pre_gt)
                # --- BN stats allreduce + scale/shift ---
                stsb = smp.tile([1, 2 * D], F32, tag="stsb")
                nc.scalar.activation(stsb[:, 0:D], stats0[:],
                                     mybir.ActivationFunctionType.Copy)
                nc.scalar.activation(stsb[:, D:2 * D], stats1[:],
                                     mybir.ActivationFunctionType.Copy)
                nc.sync.dma_start(t_arin[l][:], stsb[:])
                allreduce([t_arin[l][:]], [t_arout[l][:]])
                stg = smp.tile([1, 2 * D], F32, tag="stg")
                nc.sync.dma_start(stg[:], t_arout[l][:])
                mean = smp.tile([1, D], F32, tag="mean")
                nc.vector.tensor_scalar_mul(mean[:], stg[:, 0:D], 1.0 / NREAL)
                msq = smp.tile([1, D], F32, tag="msq")
                nc.vector.tensor_scalar_mul(msq[:], stg[:, D:2 * D],
                                            1.0 / NREAL)
                var = smp.tile([1, D], F32, tag="var")
                nc.vector.tensor_tensor(out=var[:], in0=mean[:], in1=mean[:],
                                        op=mybir.AluOpType.mult)
                nc.vector.tensor_tensor(out=var[:], in0=msq[:], in1=var[:],
                                        op=mybir.AluOpType.subtract)
                sd = smp.tile([1, D], F32, tag="sd")
                nc.scalar.activation(sd[:], var[:],
                                     mybir.ActivationFunctionType.Sqrt,
                                     bias=epsS[:])
                rsq = smp.tile([1, D], F32, tag="rsq")
                nc.vector.reciprocal(rsq[:], sd[:])
                scl = smp.tile([1, D], F32, tag="scl")
                nc.vector.tensor_tensor(out=scl[:], in0=rsq[:],
                                        in1=gamS[:, l, :],
                                        op=mybir.AluOpType.mult)
                sft = smp.tile([1, D], F32, tag="sft")
                nc.vector.tensor_tensor(out=sft[:], in0=mean[:], in1=scl[:],
                                        op=mybir.AluOpType.mult)
                nc.vector.tensor_tensor(out=sft[:], in0=betS[:, l, :],
                                        in1=sft[:],
                                        op=mybir.AluOpType.subtract)
                ab = aps.tile([P, 2 * D], F32, tag="ab")
                nc.tensor.matmul(out=ab[:, 0:D], lhsT=onesS[:], rhs=scl[:],
                                 start=True, stop=True)
                nc.tensor.matmul(out=ab[:, D:2 * D], lhsT=onesS[:], rhs=sft[:],
                                 start=True, stop=True)
                # --- apply (window-batched) + (layer L-1) pooling ---
                a0 = ab[:, 0:D]
                a0b = bass.AP(a0.tensor, a0.offset,
                              [a0.ap[0], [0, SPW], a0.ap[1]])
                a1 = ab[:, D:2 * D]
                a1b = bass.AP(a1.tensor, a1.offset,
                              [a1.ap[0], [0, SPW], a1.ap[1]])
                for w in range(NWIN):
                    hnf = wk.tile([P, SPW, D], F32, tag="hnf", bufs=2)
                    nc.vector.tensor_tensor(
                        out=hnf[:], in0=hlinS[:, w * SPW:(w + 1) * SPW, :],
                        in1=a0b, op=mybir.AluOpType.mult)
                    nc.vector.tensor_tensor(out=hnf[:], in0=hnf[:],
                                            in1=a1b, op=mybir.AluOpType.add)
                    if l < L - 1:
                        nc.scalar.activation(
                            hnbS[:, w * SPW:(w + 1) * SPW, :], hnf[:],
                            mybir.ActivationFunctionType.Relu)
                        nc.sync.dma_start(
                            t_hnew.ap()[w * WSZ:(w + 1) * WSZ, :].rearrange(
                                "(s p) d -> p s d", p=P),
                            hnbS[:, w * SPW:(w + 1) * SPW, :])
                    else:
                        hnr = wk.tile([P, SPW, D], F32, tag="hnr", bufs=2)
                        nc.vector.tensor_scalar_max(hnr[:], hnf[:], 0.0)
                        for sw in range(SPW):
                            st = w * SPW + sw
                            for h in range(KD):
                                nc.tensor.matmul(
                                    out=poolps[h][:],
                                    lhsT=hnr[:, sw, h * P:(h + 1) * P],
                                    rhs=selpS[:, st, :],
                                    start=(st == 0), stop=(st == NSW - 1))

            # ================= readout =================
            gts = smp.tile([P, KD * GPC], F32, tag="gts")
            for h in range(KD):
                nc.scalar.activation(gts[:, h * GPC:(h + 1) * GPC],
                                     poolps[h][:],
                                     mybir.ActivationFunctionType.Copy)
            ones16 = smp.tile([1, GPC], F32, tag="o16")
            nc.vector.memset(ones16[:], 1.0)
            outp = sps.tile([GPC, OUT], F32, tag="stats0")
            for h in range(KD):
                nc.tensor.matmul(out=outp[:],
                                 lhsT=gts[:, h * GPC:(h + 1) * GPC],
                                 rhs=wpS[:, h, :], start=(h == 0), stop=False)
            nc.tensor.matmul(out=outp[:], lhsT=ones16[:], rhs=bpS[:],
                             start=False, stop=True)
            outs = smp.tile([GPC, OUT], F32, tag="outs")
            nc.scalar.activation(outs[:], outp[:],
                                 mybir.ActivationFunctionType.Copy)
            nc.sync.dma_start(t_out[:], outs[:])

    nc.compile()
    return nc


LAST = {}


def kernel(**inputs):
    cfg, in_maps, _ = preprocess(inputs)
    nc = build(cfg)
    trace = os.environ.get("KGCN_TRACE") == "1"
    res = run_bass_kernel_spmd(nc, in_maps, list(range(NCORES)), trace=trace)
    LAST["exec_time_ns"] = res.exec_time_ns
    LAST["profile_json"] = res.profile_json
    out = np.concatenate([res.results[c]["out_g"] for c in range(NCORES)], 0)
    return out.astype(np.float32)


if __name__ == "__main__":
    pass


# revision 46
# speedup vs baseline: 1.4861x; 1.0555x over previous
"""GCN message-passing kernel for 8 Trainium2 NeuronCores (Bass/Tile).

v2 redesign vs v1 (4.66ms -> target <2.5ms):
- fp8 h table + Shared collective outputs by default (v1 had them off).
- Gathers merged per (window, stream): 26 DMAGatherAnt/layer instead of 104
  (amortizes ~1us fixed Q7 descriptor-gen cost per instruction).
- Selection matrices precomputed on host and DMA'd as inputs (removes the
  DVE IS_EQ chain, ~700us, plus GpSimd SBUF-port contention).
- Self-loop h term no longer gathered: post-activation h kept in SBUF
  (hnbS) and added to the PSUM window via identity matmuls (-5% descriptors).
- Bond-encoder matmuls window-wide (512-free) instead of per-subwindow.
- Dead-row zeroing dropped (pad edges have all-zero sel columns; pad nodes
  have deginv=0 and zero selpool rows, so garbage never propagates).
- AllReduce output Shared.
"""
import sys

sys.path.insert(0, "/opt/trn_rl_repo")

import os

import numpy as np
import ml_dtypes

import concourse.bass as bass
import concourse.bacc as bacc
import concourse.mybir as mybir
import concourse.tile as tile
from concourse.bass_utils import run_bass_kernel_spmd

P = 128
WSZ = 512          # psum node window
EPS = 1e-5
NCORES = 8
BF16 = mybir.dt.bfloat16
FP8 = mybir.dt.float8e4
F32 = mybir.dt.float32
I16 = mybir.dt.int16
S0 = 64.0          # layer-0 table scale (absorbed by BN)

USE_FP8 = os.environ.get("KGCN_FP8", "1") == "1"
HDT = FP8 if USE_FP8 else BF16
HNP = ml_dtypes.float8_e4m3fn if USE_FP8 else ml_dtypes.bfloat16


# ----------------------------------------------------------------------------
# Host preprocessing
# ----------------------------------------------------------------------------

def _wrap_idx(flat):
    n = flat.shape[0]
    assert n % 16 == 0
    w = flat.reshape(n // 16, 16).T.astype(np.int16)  # [16, n/16]
    return np.tile(w, (8, 1))


def preprocess(inputs, n_graphs=128):
    nfeat = np.asarray(inputs["nfeat"], np.int64)
    efeat = np.asarray(inputs["efeat"], np.int64)
    src = np.asarray(inputs["src"], np.int64)
    dst = np.asarray(inputs["dst"], np.int64)
    graph_ids = np.asarray(inputs["graph_ids"], np.int64)
    atom_emb = np.asarray(inputs["atom_emb"], np.float32)
    edge_emb = np.asarray(inputs["edge_emb"], np.float32)
    W = np.asarray(inputs["W"], np.float32)
    gamma = np.asarray(inputs["gamma"], np.float32)
    beta = np.asarray(inputs["beta"], np.float32)
    Wp = np.asarray(inputs["Wp"], np.float32)
    bp = np.asarray(inputs["bp"], np.float32)

    N = graph_ids.shape[0]
    E = src.shape[0]
    G = n_graphs
    GPC = G // NCORES
    AC, AV, D = atom_emb.shape
    L, BC, BV, _ = edge_emb.shape
    NCOMB = BV ** BC
    OUT = Wp.shape[1]
    HALF = NCORES // 2

    gcnt = np.bincount(graph_ids, minlength=G)
    gofs = np.concatenate([[0], np.cumsum(gcnt)])
    S = gofs[::GPC].astype(np.int64)
    assert S[-1] == N
    Nc = np.diff(S)

    NSW = int(np.ceil((Nc.max() + 1) / P))
    NPU = NSW * P
    NWIN = NPU // WSZ
    if NWIN * WSZ < NPU:
        NWIN += 1
        NPU = NWIN * WSZ
        NSW = NPU // P
    SPW = WSZ // P
    B_SPLIT = HALF * NPU
    assert B_SPLIT < 32768 and (NCORES - HALF) * NPU < 32768

    degs = np.bincount(dst, minlength=N).astype(np.float64) + 1.0
    deginv_all = (1.0 / degs).astype(np.float32)

    node_core = np.searchsorted(S[1:], np.arange(N), side="right").astype(np.int64)
    src_core = node_core[src]
    e_isL = src_core < HALF
    dLn = np.bincount(dst[e_isL], minlength=N)
    dHn = np.bincount(dst[~e_isL], minlength=N)

    # --- per-core node permutation: balance (dL, dH) across NSW bins ---
    pos_of_node = np.full(N, -1, np.int64)
    node_at_pos = [np.full(NPU, -1, np.int64) for _ in range(NCORES)]
    for c in range(NCORES):
        nodes = np.arange(S[c], S[c + 1])
        wl = dLn[nodes].astype(np.int64)
        wh = dHn[nodes].astype(np.int64)
        order = np.argsort(-(wl + wh), kind="stable")
        binL = np.zeros(NSW, np.int64)
        binH = np.zeros(NSW, np.int64)
        binN = np.zeros(NSW, np.int64)
        for i in order:
            nl, nh = wl[i], wh[i]
            cand = np.maximum(binL + nl, binH + nh) + 1e-3 * (binL + binH)
            cand[binN >= P] = 1 << 60
            b = int(np.argmin(cand))
            slot = binN[b]
            binN[b] += 1
            binL[b] += nl
            binH[b] += nh
            n = nodes[i]
            pos_of_node[n] = c * NPU + b * P + slot
            node_at_pos[c][b * P + slot] = n

    # --- edge streams per (core, stream): edges only, sorted by dst ---
    src_pg = pos_of_node[src]
    dst_pos = pos_of_node[dst]
    dst_core = node_core[dst]

    core_streams = []   # [core][stream] -> (srcpos_sorted, dstlocal_sorted)
    for c in range(NCORES):
        em = dst_core == c
        es, ed, eL = src_pg[em], dst_pos[em] - c * NPU, e_isL[em]
        per = {}
        for stream, m in (("L", eL), ("H", ~eL)):
            ssrc = es[m]
            sdst = ed[m]
            o = np.argsort(sdst, kind="stable")
            ssrc, sdst = ssrc[o], sdst[o]
            if stream == "H":
                ssrc = ssrc - B_SPLIT
            per[stream] = (ssrc, sdst)
        core_streams.append(per)

    # tiles per (sw, stream) = global max of ceil(edges_sw / P)
    TT = {}
    for stream in ("L", "H"):
        mx = 1
        for c in range(NCORES):
            _, sdst = core_streams[c][stream]
            cnt = np.bincount(sdst // P, minlength=NSW)
            mx = max(mx, int(np.ceil(cnt.max() / P)))
        TT[stream] = mx

    def pack_core(c):
        out = {}
        for stream in ("L", "H"):
            ssrc, sdst = core_streams[c][stream]
            tt = TT[stream]
            idx = np.zeros((NSW, tt * P), np.int16)   # pad -> row 0 (sel=0)
            sel = np.zeros((P, NSW * tt, P), np.float32)
            sw_of = sdst // P
            starts = np.concatenate([[0], np.cumsum(np.bincount(sw_of, minlength=NSW))])
            for sw in range(NSW):
                r0, r1 = starts[sw], starts[sw + 1]
                k = r1 - r0
                assert k <= tt * P
                idx[sw, :k] = ssrc[r0:r1]
                rows = np.arange(k)
                sel[rows % P, sw * tt + rows // P, sdst[r0:r1] - sw * P] = 1.0
            out[stream] = (idx.reshape(-1), sel.astype(HNP))
        return out

    packed = [pack_core(c) for c in range(NCORES)]

    # --- tables ---
    # atom9[p, a, :] = atom_emb[a, p, :] * S0  (vocab entry p of column a)
    atom9_q = (np.transpose(atom_emb, (1, 0, 2)) * S0).astype(HNP)

    k = np.arange(NCOMB)
    d0, d1, d2 = k // (BV * BV), (k // BV) % BV, k % BV
    T512 = edge_emb[:, 0, d0] + edge_emb[:, 1, d1] + edge_emb[:, 2, d2]
    T512[0] *= S0
    T512_q = T512.astype(HNP)

    cidx = (efeat[:, 0] * BV + efeat[:, 1]) * BV + efeat[:, 2]

    cfg = dict(N=N, E=E, G=G, GPC=GPC, D=D, L=L, OUT=OUT, NPU=NPU, NSW=NSW,
               NWIN=NWIN, SPW=SPW, TT_L=TT["L"], TT_H=TT["H"],
               B_SPLIT=B_SPLIT, NCOMB=NCOMB, AC=AC, NREAL=N)

    in_maps = []
    for c in range(NCORES):
        m = {}
        for stream in ("L", "H"):
            idx, sel = packed[c][stream]
            m[f"gidx{stream}"] = _wrap_idx(idx)
            m[f"sel{stream}"] = sel.reshape(P, -1).copy()
        em = dst_core == c
        lp = dst_pos[em] - c * NPU
        ct = np.zeros((NCOMB, NPU), np.float32)
        np.add.at(ct, (cidx[em], lp), 1.0)
        m["countT"] = ct.astype(HNP)
        dg = np.zeros(NPU, np.float32)
        rp = node_at_pos[c] >= 0
        dg[rp] = deginv_all[node_at_pos[c][rp]]
        m["deginv"] = np.tile(dg[None, :], (P, 1)).astype(ml_dtypes.bfloat16)
        sp = np.zeros((NPU, GPC), np.float32)
        gl = np.where(rp)[0]
        gid = graph_ids[node_at_pos[c][gl]] - c * GPC
        cnts = np.maximum(gcnt[c * GPC:(c + 1) * GPC], 1.0)
        sp[gl, gid] = (1.0 / cnts[gid]).astype(np.float32)
        m["selpool"] = sp
        cnt9 = np.zeros((P, AC, NPU), HNP)
        pos_r = np.where(rp)[0]
        nf = nfeat[node_at_pos[c][pos_r]]
        for a in range(AC):
            cnt9[nf[:, a], a, pos_r] = 1.0
        m["cnt9"] = cnt9.reshape(P, -1).copy()
        m["atom9"] = atom9_q
        m["t512"] = T512_q
        m["wl"] = W.astype(ml_dtypes.bfloat16)             # [L, D, D]
        m["gam"] = gamma.reshape(L, 1, D).copy()
        m["bet"] = beta.reshape(L, 1, D).copy()
        m["wp"] = Wp.copy()
        m["bpr"] = bp.reshape(1, OUT).copy()
        m["ident"] = np.eye(P, dtype=HNP)
        m["dzero"] = np.zeros((P, P // 16), np.int16)
        in_maps.append(m)

    meta = dict(S=S, Nc=Nc)
    return cfg, in_maps, meta


# ----------------------------------------------------------------------------
# Device kernel builder (uniform SPMD program)
# ----------------------------------------------------------------------------

def build(cfg):
    D = cfg["D"]; L = cfg["L"]; NPU = cfg["NPU"]; NSW = cfg["NSW"]
    NWIN = cfg["NWIN"]; SPW = cfg["SPW"]; TT_L = cfg["TT_L"]; TT_H = cfg["TT_H"]
    NCOMB = cfg["NCOMB"]; AC = cfg["AC"]; GPC = cfg["GPC"]; OUT = cfg["OUT"]
    B_SPLIT = cfg["B_SPLIT"]; NREAL = cfg["NREAL"]
    KD = D // P
    NKC = NCOMB // P
    NT_L, NT_H = NSW * TT_L, NSW * TT_H
    WT_L, WT_H = SPW * TT_L, SPW * TT_H     # gather tiles per window
    NQ = int(os.environ.get("KGCN_NQ", "4"))
    PREP = os.environ.get("KGCN_PREP", "0") == "1"
    NPRE = 3                                 # windows prepped ahead at layer start

    nc = bacc.Bacc("TRN2", target_bir_lowering=False, debug=False,
                   num_devices=NCORES, num_swdge_queues=NQ)

    def allgather(ins, outs):
        if USE_FP8:
            ins = [ap.bitcast(BF16) for ap in ins]
            outs = [ap.bitcast(BF16) for ap in outs]
        nc.gpsimd.collective_compute(
            "AllGather", mybir.AluOpType.bypass,
            replica_groups=[list(range(NCORES))], ins=ins, outs=outs)

    def allreduce(ins, outs):
        nc.gpsimd.collective_compute(
            "AllReduce", mybir.AluOpType.add,
            replica_groups=[list(range(NCORES))], ins=ins, outs=outs)

    t_gidxL = nc.dram_tensor("gidxL", [P, NT_L * P // 16], I16, kind="ExternalInput")
    t_gidxH = nc.dram_tensor("gidxH", [P, NT_H * P // 16], I16, kind="ExternalInput")
    t_selL = nc.dram_tensor("selL", [P, NT_L * P], HDT, kind="ExternalInput")
    t_selH = nc.dram_tensor("selH", [P, NT_H * P], HDT, kind="ExternalInput")
    t_countT = nc.dram_tensor("countT", [NCOMB, NPU], HDT, kind="ExternalInput")
    t_deginv = nc.dram_tensor("deginv", [P, NPU], BF16, kind="ExternalInput")
    t_selpool = nc.dram_tensor("selpool", [NPU, GPC], F32, kind="ExternalInput")
    t_cnt9 = nc.dram_tensor("cnt9", [P, AC * NPU], HDT, kind="ExternalInput")
    t_atom9 = nc.dram_tensor("atom9", [P, AC, D], HDT, kind="ExternalInput")
    t_t512 = nc.dram_tensor("t512", [L, NCOMB, D], HDT, kind="ExternalInput")
    t_wl = nc.dram_tensor("wl", [L, D, D], BF16, kind="ExternalInput")
    t_gam = nc.dram_tensor("gam", [L, 1, D], F32, kind="ExternalInput")
    t_bet = nc.dram_tensor("bet", [L, 1, D], F32, kind="ExternalInput")
    t_wp = nc.dram_tensor("wp", [D, OUT], F32, kind="ExternalInput")
    t_bp = nc.dram_tensor("bpr", [1, OUT], F32, kind="ExternalInput")
    t_ident = nc.dram_tensor("ident", [P, P], HDT, kind="ExternalInput")
    t_dzero = nc.dram_tensor("dzero", [P, P // 16], I16, kind="ExternalInput")
    t_out = nc.dram_tensor("out_g", [GPC, OUT], F32, kind="ExternalOutput")
    t_hfull = nc.dram_tensor("h_full", [NCORES * NPU, D], HDT, addr_space="Shared")
    t_hnew = nc.dram_tensor("h_newc", [NPU, D], HDT)
    t_arin = [nc.dram_tensor(f"arin{l}", [1, 2 * D], F32) for l in range(L)]
    t_arout = [nc.dram_tensor(f"arout{l}", [1, 2 * D], F32,
                              addr_space="Shared") for l in range(L)]

    dma_sems = [nc.alloc_semaphore(f"swdge_dma{q}") for q in range(NQ)]

    def wqueues(w):
        qa = (2 * w) % NQ
        return qa, qa + 1

    with tile.TileContext(nc) as tc:
        with (
            tc.tile_pool(name="static", bufs=1) as stp,
            tc.tile_pool(name="gath", bufs=4) as gpool,
            tc.tile_pool(name="selp", bufs=2) as selpool_p,
            tc.tile_pool(name="xt", bufs=2) as xtp,
            tc.tile_pool(name="work", bufs=3) as wk,
            tc.tile_pool(name="small", bufs=1) as smp,
            tc.tile_pool(name="winps", bufs=2, space="PSUM") as wps,
            tc.tile_pool(name="hlps", bufs=1, space="PSUM") as hps,
            tc.tile_pool(name="smps", bufs=1, space="PSUM") as sps,
            tc.tile_pool(name="abps", bufs=1, space="PSUM") as aps,
        ):
            # ---- static SBUF preloads ----
            atom9S = stp.tile([P, AC, D], HDT)
            selpS = stp.tile([P, NSW, GPC], F32)
            dgS = stp.tile([P, NPU], BF16)
            wS = stp.tile([P, L, KD, D], BF16)
            t5S = stp.tile([P, L, NKC, D], HDT)
            gamS = stp.tile([1, L, D], F32)
            betS = stp.tile([1, L, D], F32)
            wpS = stp.tile([P, KD, OUT], F32)
            bpS = stp.tile([1, OUT], F32)
            onesS = stp.tile([1, P], F32)
            onecol = stp.tile([P, 1], BF16)
            identS = stp.tile([P, P], HDT)
            hlinS = stp.tile([P, NSW, D], BF16)
            hnbS = stp.tile([P, NSW, D], HDT)
            epsS = stp.tile([1, 1], F32)
            nc.vector.memset(epsS[:], EPS)
            nc.sync.dma_start(atom9S[:], t_atom9[:])
            nc.sync.dma_start(identS[:], t_ident[:])
            nc.sync.dma_start(selpS[:], t_selpool.ap().rearrange("(s p) g -> p s g", p=P))
            nc.sync.dma_start(dgS[:], t_deginv[:])
            nc.sync.dma_start(wS[:], t_wl.ap().rearrange("l (k p) d -> p l k d", p=P))
            nc.sync.dma_start(t5S[:], t_t512.ap().rearrange("l (k p) d -> p l k d", p=P))
            nc.sync.dma_start(gamS[:], t_gam.ap().rearrange("l o d -> o l d"))
            nc.sync.dma_start(betS[:], t_bet.ap().rearrange("l o d -> o l d"))
            nc.sync.dma_start(wpS[:], t_wp.ap().rearrange("(k p) o -> p k o", p=P))
            nc.sync.dma_start(bpS[:], t_bp[:])
            nc.vector.memset(onesS[:], 1.0)
            nc.vector.memset(onecol[:], 1.0)
            dzeroS = stp.tile([P, P // 16], I16)
            nc.sync.dma_start(dzeroS[:], t_dzero[:])

            # ============ h0: atom embedding sums via count matmuls ============
            for w in range(NWIN):
                cnt = wk.tile([P, AC, WSZ], HDT, tag="cnt", bufs=2)
                nc.sync.dma_start(
                    cnt[:], t_cnt9.ap().rearrange("p (a n) -> p a n", a=AC)
                    [:, :, w * WSZ:(w + 1) * WSZ])
                for sw in range(SPW):
                    st = w * SPW + sw
                    h0p = hps.tile([P, D], F32, tag="hl")
                    for a in range(AC):
                        nc.tensor.matmul(
                            out=h0p[:],
                            lhsT=cnt[:, a, sw * P:(sw + 1) * P],
                            rhs=atom9S[:, a, :],
                            start=(a == 0), stop=(a == AC - 1))
                    nc.scalar.activation(hnbS[:, st, :], h0p[:],
                                         mybir.ActivationFunctionType.Copy)
                    nc.sync.dma_start(t_hnew[st * P:(st + 1) * P, :],
                                      hnbS[:, st, :])

            # ================= layers =================
            def emit_prep(w, stream, gt, prep=True):
                wt = WT_L if stream == "L" else WT_H
                tg = t_gidxL if stream == "L" else t_gidxH
                nidx = wt * P
                gidx = gpool.tile([P, nidx // 16], I16, tag=f"i{stream}",
                                  name=f"i{stream}t")
                nc.sync.dma_start(
                    gidx[:], tg[:, w * (nidx // 16):(w + 1) * (nidx // 16)])
                tbl = (t_hfull[0:B_SPLIT, :] if stream == "L"
                       else t_hfull[B_SPLIT:NCORES * NPU, :])
                qa, qb = wqueues(w)
                q = qa if stream == "L" else qb
                if prep:
                    nc.gpsimd.dma_gather(
                        gt[:], tbl, gidx[:],
                        nidx, nidx, D, single_packet=False,
                        prepare_only=True, sem=dma_sems[q], queue_num=q)
                else:
                    nc.gpsimd.dma_gather(
                        gt[:], tbl, gidx[:],
                        nidx, nidx, D, single_packet=False, queue_num=q)
                return q

            def new_gt(stream):
                wt = WT_L if stream == "L" else WT_H
                return gpool.tile([P, wt, D], HDT, tag=f"g{stream}",
                                  name=f"g{stream}t")

            def new_sel(w, stream):
                wt = WT_L if stream == "L" else WT_H
                tsel = t_selL if stream == "L" else t_selH
                sel = selpool_p.tile([P, wt, P], HDT, tag=f"s{stream}",
                                     name=f"s{stream}t")
                nc.sync.dma_start(
                    sel[:], tsel[:, w * (wt * P):(w + 1) * (wt * P)])
                return sel

            def new_ctk(w):
                ctk = wk.tile([P, NKC, WSZ], HDT, tag="ct", bufs=2)
                nc.sync.dma_start(
                    ctk[:], t_countT.ap().rearrange(
                        "(k p) n -> p k n", p=P)[:, :, w * WSZ:(w + 1) * WSZ])
                return ctk

            def shadow_preps(pre_gt):
                # Shadow preps for the NEXT layer's first windows: traced
                # before the BN stats/AR/apply tail, so the Pool engine
                # generates their descriptors during that tail.  The ring
                # entries stay untriggered until fire_shadow().
                if not PREP:
                    return
                for w in range(NPRE):
                    for stream in ("L", "H"):
                        gt = new_gt(stream)
                        emit_prep(w, stream, gt)
                        pre_gt[(w, stream)] = gt

            def fire_shadow():
                # Dummy preps traced AFTER the AllGather: their deferred
                # h_full read binds the triggers to the fresh table, gating
                # the shadow preps' DMAs correctly.
                for q in range(NQ):
                    dgt = gpool.tile([P, 1, D], HDT, tag="gd", name="gdt",
                                     bufs=2)
                    nc.gpsimd.dma_gather(
                        dgt[:], t_hfull[0:B_SPLIT, :], dzeroS[:], P, P, D,
                        single_packet=False, prepare_only=True,
                        sem=dma_sems[q], queue_num=q)
                for q in range(NQ):
                    nc.gpsimd.trigger_dma(count=None, queue_num=q)

            pre_gt = {}
            shadow_preps(pre_gt)

            for l in range(L):
                allgather([t_hnew[:]], [t_hfull[:]])
                if pre_gt:
                    fire_shadow()
                pre_sel = {}
                pre_ctk = {}
                for w in range(NPRE - 1):
                    pre_ctk[w] = new_ctk(w)
                    for stream in ("L", "H"):
                        pre_sel[(w, stream)] = new_sel(w, stream)
                stats0 = sps.tile([1, D], F32, tag="stats0")
                stats1 = sps.tile([1, D], F32, tag="stats1")
                if l == L - 1:
                    poolps = [sps.tile([P, GPC], F32, tag=f"pool{h}",
                                       name=f"pool{h}") for h in range(KD)]
                for w in range(NWIN):
                    winp = [wps.tile([P, WSZ], F32, tag="win", name=f"win{h}")
                            for h in range(KD)]
                    ctk = pre_ctk.pop(w) if w in pre_ctk else new_ctk(w)
                    gts = {}
                    sels = {}
                    for stream in ("L", "H"):
                        if (w, stream) in pre_gt:
                            gts[stream] = pre_gt.pop((w, stream))
                        else:
                            gt = new_gt(stream)
                            emit_prep(w, stream, gt, prep=False)
                            gts[stream] = gt
                        if (w, stream) in pre_sel:
                            sels[stream] = pre_sel.pop((w, stream))
                        else:
                            sels[stream] = new_sel(w, stream)
                    # bond term: window-wide, starts the PSUM accumulation
                    for kk in range(NKC):
                        for h in range(KD):
                            nc.tensor.matmul(
                                out=winp[h][:],
                                lhsT=t5S[:, l, kk, h * P:(h + 1) * P],
                                rhs=ctk[:, kk, :],
                                start=(kk == 0), stop=False)
                    for sw in range(SPW):
                        st = w * SPW + sw
                        # self term via identity (hnbS holds this layer's input)
                        for h in range(KD):
                            nc.tensor.matmul(
                                out=winp[h][:, sw * P:(sw + 1) * P],
                                lhsT=hnbS[:, st, h * P:(h + 1) * P],
                                rhs=identS[:],
                                start=False, stop=False)
                        for stream, tt in (("L", TT_L), ("H", TT_H)):
                            gt = gts[stream]
                            sel = sels[stream]
                            last_stream = stream == "H"
                            for t in range(tt):
                                ti = sw * tt + t
                                for h in range(KD):
                                    nc.tensor.matmul(
                                        out=winp[h][:, sw * P:(sw + 1) * P],
                                        lhsT=gt[:, ti, h * P:(h + 1) * P],
                                        rhs=sel[:, ti, :],
                                        start=False,
                                        stop=(last_stream and t == tt - 1))
                    # x^T = deginv * window  (bf16)
                    xt = [xtp.tile([P, WSZ], BF16, tag="xt", name=f"xt{h}")
                          for h in range(KD)]
                    for h in range(KD):
                        nc.vector.tensor_tensor(
                            out=xt[h][:], in0=winp[h][:],
                            in1=dgS[:, w * WSZ:(w + 1) * WSZ],
                            op=mybir.AluOpType.mult)
                    # update matmul + stats per subtile
                    for sw in range(SPW):
                        st = w * SPW + sw
                        hlp = hps.tile([P, D], F32, tag="hl")
                        for h in range(KD):
                            nc.tensor.matmul(
                                out=hlp[:],
                                lhsT=xt[h][:, sw * P:(sw + 1) * P],
                                rhs=wS[:, l, h, :],
                                start=(h == 0), stop=(h == KD - 1))
                        nc.scalar.activation(hlinS[:, st, :], hlp[:],
                                             mybir.ActivationFunctionType.Copy)
                        sq = wk.tile([P, D], BF16, tag="sq")
                        nc.vector.tensor_tensor(out=sq[:], in0=hlinS[:, st, :],
                                                in1=hlinS[:, st, :],
                                                op=mybir.AluOpType.mult)
                        nc.tensor.matmul(out=stats0[:],
                                         lhsT=onecol[:],
                                         rhs=hlinS[:, st, :],
                                         start=(st == 0), stop=(st == NSW - 1))
                        nc.tensor.matmul(out=stats1[:],
                                         lhsT=onecol[:], rhs=sq[:],
                                         start=(st == 0), stop=(st == NSW - 1))
                if l < L - 1:
                    shadow_preps(pre_gt)
                # --- BN stats allreduce + scale/shift ---
                stsb = smp.tile([1, 2 * D], F32, tag="stsb")
                nc.scalar.activation(stsb[:, 0:D], stats0[:],
                                     mybir.ActivationFunctionType.Copy)
                nc.scalar.activation(stsb[:, D:2 * D], stats1[:],
                                     mybir.ActivationFunctionType.Copy)
                nc.sync.dma_start(t_arin[l][:], stsb[:])
                allreduce([t_arin[l][:]], [t_arout[l][:]])
                stg = smp.tile([1, 2 * D], F32, tag="stg")
                nc.sync.dma_start(stg[:], t_arout[l][:])
                mean = smp.tile([1, D], F32, tag="mean")
                nc.vector.tensor_scalar_mul(mean[:], stg[:, 0:D], 1.0 / NREAL)
                msq = smp.tile([1, D], F32, tag="msq")
                nc.vector.tensor_scalar_mul(msq[:], stg[:, D:2 * D],
                                            1.0 / NREAL)
                var = smp.tile([1, D], F32, tag="var")
                nc.vector.tensor_tensor(out=var[:], in0=mean[:], in1=mean[:],
                                        op=mybir.AluOpType.mult)
                nc.vector.tensor_tensor(out=var[:], in0=msq[:], in1=var[:],
                                        op=mybir.AluOpType.subtract)
                sd = smp.tile([1, D], F32, tag="sd")
                nc.scalar.activation(sd[:], var[:],
                                     mybir.ActivationFunctionType.Sqrt,
                                     bias=epsS[:])
                rsq = smp.tile([1, D], F32, tag="rsq")
                nc.vector.reciprocal(rsq[:], sd[:])
                scl = smp.tile([1, D], F32, tag="scl")
                nc.vector.tensor_tensor(out=scl[:], in0=rsq[:],
                                        in1=gamS[:, l, :],
                                        op=mybir.AluOpType.mult)
                sft = smp.tile([1, D], F32, tag="sft")
                nc.vector.tensor_tensor(out=sft[:], in0=mean[:], in1=scl[:],
                                        op=mybir.AluOpType.mult)
                nc.vector.tensor_tensor(out=sft[:], in0=betS[:, l, :],
                                        in1=sft[:],
                                        op=mybir.AluOpType.subtract)
                ab = aps.tile([P, 2 * D], F32, tag="ab")
                nc.tensor.matmul(out=ab[:, 0:D], lhsT=onesS[:], rhs=scl[:],
                                 start=True, stop=True)
                nc.tensor.matmul(out=ab[:, D:2 * D], lhsT=onesS[:], rhs=sft[:],
                                 start=True, stop=True)
                # --- apply (window-batched) + (layer L-1) pooling ---
                a0 = ab[:, 0:D]
                a0b = bass.AP(a0.tensor, a0.offset,
                              [a0.ap[0], [0, SPW], a0.ap[1]])
                a1 = ab[:, D:2 * D]
                a1b = bass.AP(a1.tensor, a1.offset,
                              [a1.ap[0], [0, SPW], a1.ap[1]])
                for w in range(NWIN):
                    hnf = wk.tile([P, SPW, D], F32, tag="hnf", bufs=2)
                    nc.vector.tensor_tensor(
                        out=hnf[:], in0=hlinS[:, w * SPW:(w + 1) * SPW, :],
                        in1=a0b, op=mybir.AluOpType.mult)
                    nc.vector.tensor_tensor(out=hnf[:], in0=hnf[:],
                                            in1=a1b, op=mybir.AluOpType.add)
                    if l < L - 1:
                        nc.scalar.activation(
                            hnbS[:, w * SPW:(w + 1) * SPW, :], hnf[:],
                            mybir.ActivationFunctionType.Relu)
                        nc.sync.dma_start(
                            t_hnew.ap()[w * WSZ:(w + 1) * WSZ, :].rearrange(
                                "(s p) d -> p s d", p=P),
                            hnbS[:, w * SPW:(w + 1) * SPW, :])
                    else:
                        hnr = wk.tile([P, SPW, D], F32, tag="hnr", bufs=1)
                        nc.vector.tensor_scalar_max(hnr[:], hnf[:], 0.0)
                        for sw in range(SPW):
                            st = w * SPW + sw
                            for h in range(KD):
                                nc.tensor.matmul(
                                    out=poolps[h][:],
                                    lhsT=hnr[:, sw, h * P:(h + 1) * P],
                                    rhs=selpS[:, st, :],
                                    start=(st == 0), stop=(st == NSW - 1))

            # ================= readout =================
            gts = smp.tile([P, KD * GPC], F32, tag="gts")
            for h in range(KD):
                nc.scalar.activation(gts[:, h * GPC:(h + 1) * GPC],
                                     poolps[h][:],
                                     mybir.ActivationFunctionType.Copy)
            ones16 = smp.tile([1, GPC], F32, tag="o16")
            nc.vector.memset(ones16[:], 1.0)
            outp = sps.tile([GPC, OUT], F32, tag="stats0")
            for h in range(KD):
                nc.tensor.matmul(out=outp[:],
                                 lhsT=gts[:, h * GPC:(h + 1) * GPC],
                                 rhs=wpS[:, h, :], start=(h == 0), stop=False)
            nc.tensor.matmul(out=outp[:], lhsT=ones16[:], rhs=bpS[:],
                             start=False, stop=True)
            outs = smp.tile([GPC, OUT], F32, tag="outs")
            nc.scalar.activation(outs[:], outp[:],
                                 mybir.ActivationFunctionType.Copy)
            nc.sync.dma_start(t_out[:], outs[:])

    nc.compile()
    return nc


LAST = {}


def kernel(**inputs):
    cfg, in_maps, _ = preprocess(inputs)
    nc = build(cfg)
    trace = os.environ.get("KGCN_TRACE") == "1"
    res = run_bass_kernel_spmd(nc, in_maps, list(range(NCORES)), trace=trace)
    LAST["exec_time_ns"] = res.exec_time_ns
    LAST["profile_json"] = res.profile_json
    out = np.concatenate([res.results[c]["out_g"] for c in range(NCORES)], 0)
    return out.astype(np.float32)


if __name__ == "__main__":
    pass
